# revision 1
# baseline (speedup 1.0000x reference)
"""GCN (3-layer GCNConv + global_add_pool + linear head) on 8 Trainium2 cores.

Strategy:
 - Nodes sharded across 8 cores on graph-id-aligned boundaries (pooling local).
 - Edges partitioned by dst owner. Per core, edges ordered chunk-major
   (src_row % 4 -> int16 gather index fits), then by 128-node dst window,
   padded per (chunk, window) group to multiples of 128 and uniform tile
   counts across cores (single SPMD program).
 - Per layer: dense z = h @ W on PE (transpose-on-the-fly), zn = z * dinv,
   AllGather zn -> full table in DRAM, dma_gather 256B rows per edge,
   segment-sum via one-hot matmul into PSUM per window, accumulated in SBUF
   across the 4 chunk passes, then tanh(dinv*acc + b) in place.
 - Pooling: one-hot(graph id) matmul into a [64, 512] PSUM tile; final
   linear head + tanh on device.
"""

import hashlib
import sys

for _p in ("/opt/trn_rl_repo",):
    if _p not in sys.path:
        sys.path.insert(0, _p)

import numpy as np

P = 128
WIN = 128          # dst-window width (nodes)
NCHUNK = 4         # src chunks (int16 index limit: 8S/4 <= 32767)
GRANULE = 4096     # idxs per dma_gather instruction (ring: 3 in flight)
N_CORES = 8
N_GRAPHS = 2048    # problem constant
GMAX = 512         # per-core graph-count upper bound (psum free dim)


# ----------------------------------------------------------------------------
# Host-side sharding / edge bucketing (index manipulation only, no float math)
# ----------------------------------------------------------------------------

def _preprocess(edge_index, batch, n_nodes, n_graphs):
    C = N_CORES
    src = np.asarray(edge_index[0], dtype=np.int64)
    dst = np.asarray(edge_index[1], dtype=np.int64)
    batch = np.asarray(batch, dtype=np.int64)
    N = n_nodes

    # graph-aligned node shard boundaries
    gstart = np.searchsorted(batch, np.arange(n_graphs + 1))  # [G+1], gstart[G] = N
    node_bnds = [0]
    g_bnds = [0]
    for c in range(1, C):
        tgt = (c * N) // C
        g = int(np.searchsorted(gstart, tgt))
        # candidates g-1, g: pick nearest boundary node
        if g > 0 and abs(int(gstart[g - 1]) - tgt) <= abs(int(gstart[min(g, n_graphs)]) - tgt):
            g = g - 1
        g = min(max(g, g_bnds[-1]), n_graphs)
        g_bnds.append(g)
        node_bnds.append(int(gstart[g]))
    node_bnds.append(N)
    g_bnds.append(n_graphs)
    node_bnds = np.array(node_bnds, dtype=np.int64)          # [C+1]
    g_bnds = np.array(g_bnds, dtype=np.int64)                # [C+1]
    node_cnt = node_bnds[1:] - node_bnds[:-1]
    g_cnt = g_bnds[1:] - g_bnds[:-1]
    assert g_cnt.max() < GMAX - 1, g_cnt

    S = int(-(-node_cnt.max() // P) * P)                     # padded shard size
    NW = S // WIN                                            # windows per core
    assert 2 * S <= 32767, S                                 # int16 gather idx bound

    owner = np.searchsorted(node_bnds[1:], np.arange(N), side="right")
    local = np.arange(N) - node_bnds[owner]
    row = owner * S + local                                  # table row per node

    deg = np.bincount(dst, minlength=N).astype(np.float32) + 1.0

    # edge stream (+ self loops)
    e_src = np.concatenate([src, np.arange(N)])
    e_dst = np.concatenate([dst, np.arange(N)])
    e_owner = owner[e_dst]
    e_dl = local[e_dst]
    e_row = row[e_src]
    e_chunk = (e_row & (NCHUNK - 1)).astype(np.int64)
    e_idx = (e_row >> 2).astype(np.int16)
    e_win = e_dl >> 7

    key = (e_chunk * NW + e_win) * C + e_owner               # chunk-major, then window
    order = np.argsort(key, kind="stable")
    cnt = np.bincount(key, minlength=NCHUNK * NW * C).reshape(NCHUNK, NW, C)

    tiles_kw = -(-cnt.max(axis=2) // P)                      # [NCHUNK, NW] uniform tiles
    # chunk-0 pass initializes the SBUF accumulator (copy): force >=1 tile/window
    tiles_kw[0] = np.maximum(tiles_kw[0], 1)
    pad_kw = tiles_kw * P                                    # padded group sizes
    E_PAD = int(pad_kw.sum())
    # group start offsets in the uniform stream (same for all cores)
    goff = np.zeros((NCHUNK, NW), dtype=np.int64)
    goff.flat[1:] = np.cumsum(pad_kw.flat)[:-1]

    idx16 = np.zeros((C, E_PAD), dtype=np.int16)             # pad -> idx 0 (valid row)
    dstl = np.full((C, E_PAD), -1.0, dtype=np.float32)       # pad -> -1 (one-hot miss)

    # place real edges
    so = order
    r_owner = e_owner[so]
    r_chunk = e_chunk[so]
    r_win = e_win[so]
    # position within (chunk, win, owner) group = running index
    rkey = (r_chunk * NW + r_win) * C + r_owner
    # stable sort => positions are 0..cnt-1 in order of appearance
    pos = np.zeros(len(so), dtype=np.int64)
    _, first_idx, inv = np.unique(rkey, return_index=True, return_inverse=True)
    pos = np.arange(len(so)) - first_idx[inv]
    slot = goff[r_chunk, r_win] + pos
    idx16[r_owner, slot] = e_idx[so]
    dstl[r_owner, slot] = (e_dl[so] - r_win * WIN).astype(np.float32)

    # per-tile metadata (uniform): window id, group-first, group-last
    TILES = E_PAD // P
    tile_win = np.zeros(TILES, dtype=np.int64)
    tile_first = np.zeros(TILES, dtype=bool)
    tile_last = np.zeros(TILES, dtype=bool)
    for k in range(NCHUNK):
        for w in range(NW):
            t0 = goff[k, w] // P
            nt = int(tiles_kw[k, w])
            if nt == 0:
                continue
            tile_win[t0:t0 + nt] = w
            tile_first[t0] = True
            tile_last[t0 + nt - 1] = True
    # chunk segment boundaries (in idx positions)
    chunk_off = [int(goff[k, 0]) for k in range(NCHUNK)] + [E_PAD]

    # gather-layout idx: [16, E_PAD/16] with [p, s] = stream[s*16+p]
    idx_wrapped = np.ascontiguousarray(
        idx16.reshape(C, E_PAD // 16, 16).transpose(0, 2, 1))
    # dstl layout [128, E_PAD/128] with [p, t] = stream[t*128+p]
    dstl_wrapped = np.ascontiguousarray(
        dstl.reshape(C, TILES, P).transpose(0, 2, 1))

    # per-core padded node arrays
    deg_pad = np.ones((C, S), dtype=np.float32)
    batchl = np.full((C, S), float(GMAX - 1), dtype=np.float32)
    for c in range(C):
        n0, n1 = int(node_bnds[c]), int(node_bnds[c + 1])
        deg_pad[c, : n1 - n0] = deg[n0:n1]
        batchl[c, : n1 - n0] = (batch[n0:n1] - g_bnds[c]).astype(np.float32)
    batchl_wrapped = np.ascontiguousarray(
        batchl.reshape(C, NW, P).transpose(0, 2, 1))         # [C, 128, NW]

    return dict(
        S=S, NW=NW, E_PAD=E_PAD, TILES=TILES,
        node_bnds=node_bnds, g_bnds=g_bnds, node_cnt=node_cnt, g_cnt=g_cnt,
        idx_wrapped=idx_wrapped, dstl_wrapped=dstl_wrapped,
        batchl_wrapped=batchl_wrapped, deg_pad=deg_pad,
        tile_win=tile_win, tile_first=tile_first, tile_last=tile_last,
        chunk_off=chunk_off,
    )


# ----------------------------------------------------------------------------
# Bass program builder
# ----------------------------------------------------------------------------

def _build_program(meta, d_in, h_dim, n_cls):
    import concourse.bacc as bacc
    import concourse.mybir as mybir
    import concourse.tile as tile
    from concourse import library_config

    S, NW, E_PAD = meta["S"], meta["NW"], meta["E_PAD"]
    tile_win = meta["tile_win"]
    tile_first = meta["tile_first"]
    tile_last = meta["tile_last"]
    chunk_off = meta["chunk_off"]
    f32 = mybir.dt.float32
    AOT = mybir.ActivationFunctionType
    ALU = mybir.AluOpType

    nc = bacc.Bacc("TRN2", target_bir_lowering=False, debug=False,
                   num_devices=N_CORES)

    # --- I/O ---
    x_d = nc.dram_tensor("x_loc", [S, d_in], f32, kind="ExternalInput").ap()
    deg_d = nc.dram_tensor("deg_loc", [S], f32, kind="ExternalInput").ap()
    idx_d = nc.dram_tensor("idx16", [P, E_PAD // 16], mybir.dt.int16,
                           kind="ExternalInput").ap()
    dstl_d = nc.dram_tensor("dstl", [P, E_PAD // P], f32,
                            kind="ExternalInput").ap()
    batchl_d = nc.dram_tensor("batchl", [P, NW], f32, kind="ExternalInput").ap()
    W_d = [nc.dram_tensor("W1", [d_in, h_dim], f32, kind="ExternalInput").ap(),
           nc.dram_tensor("W2", [h_dim, h_dim], f32, kind="ExternalInput").ap(),
           nc.dram_tensor("W3", [h_dim, h_dim], f32, kind="ExternalInput").ap()]
    Wf_d = nc.dram_tensor("Wf", [h_dim, n_cls], f32, kind="ExternalInput").ap()
    b_d = [nc.dram_tensor(f"b{i+1}b", [P, h_dim], f32, kind="ExternalInput").ap()
           for i in range(3)]
    bf_d = nc.dram_tensor("bfb", [P, n_cls], f32, kind="ExternalInput").ap()
    out_d = nc.dram_tensor("out", [GMAX, n_cls], f32, kind="ExternalOutput").ap()
    ident_d = nc.dram_tensor("ident", [P, P], f32, kind="ExternalInput").ap()
    iota_w_d = nc.dram_tensor("iota_w", [P, WIN], f32, kind="ExternalInput").ap()
    iota_g_d = nc.dram_tensor("iota_g", [P, GMAX], f32, kind="ExternalInput").ap()

    zn_d = nc.dram_tensor("zn_loc", [S, h_dim], f32).ap()
    table_d = nc.dram_tensor("table", [N_CORES * S, h_dim], f32,
                             addr_space="Shared").ap()
    chunk_views = table_d.rearrange("(n four) d -> four n d", four=NCHUNK)
    rg = [list(range(N_CORES))]

    with tile.TileContext(nc) as tc:
        with (
            tc.tile_pool(name="persist", bufs=1) as pp,
            tc.tile_pool(name="msg", bufs=4) as msgp,
            tc.tile_pool(name="work", bufs=4) as wp,
            tc.tile_pool(name="dense", bufs=3) as dp,
            tc.tile_pool(name="psum", bufs=2, space="PSUM") as psp,
            tc.tile_pool(name="psum1", bufs=2, space="PSUM") as ps1,
            tc.tile_pool(name="pool_ps", bufs=1, space="PSUM") as poolps,
        ):
            # --- persistent tiles ---
            nc.gpsimd.load_library(library_config.mlp)
            ident = pp.tile([P, P], f32, tag="ident")
            nc.sync.dma_start(ident[:], ident_d[:])
            iota_w = pp.tile([P, WIN], f32, tag="iota_w")
            nc.sync.dma_start(iota_w[:], iota_w_d[:])
            iota_g = pp.tile([P, GMAX], f32, tag="iota_g")
            nc.sync.dma_start(iota_g[:], iota_g_d[:])

            W_sb = []
            for i in range(3):
                k = d_in if i == 0 else h_dim
                t = pp.tile([k, h_dim], f32, tag=f"W{i}")
                nc.sync.dma_start(t[:], W_d[i][:])
                W_sb.append(t)
            Wf_sb = pp.tile([h_dim, n_cls], f32, tag="Wf")
            nc.sync.dma_start(Wf_sb[:], Wf_d[:])
            b_sb = []
            for i in range(3):
                t = pp.tile([P, h_dim], f32, tag=f"b{i}")
                nc.sync.dma_start(t[:], b_d[i][:])
                b_sb.append(t)
            bf_sb = pp.tile([P, n_cls], f32, tag="bf")
            nc.sync.dma_start(bf_sb[:], bf_d[:])

            idx_sb = pp.tile([P, E_PAD // 16], mybir.dt.int16, tag="idx")
            nc.sync.dma_start(idx_sb[:], idx_d[:])
            dstl_sb = pp.tile([P, E_PAD // P], f32, tag="dstl")
            nc.sync.dma_start(dstl_sb[:], dstl_d[:])
            batchl_sb = pp.tile([P, NW], f32, tag="batchl")
            nc.sync.dma_start(batchl_sb[:], batchl_d[:])

            dinv = pp.tile([P, NW], f32, tag="dinv")
            deg_col = pp.tile([P, NW], f32, tag="degc")
            nc.sync.dma_start(deg_col[:], deg_d.rearrange("(t p) -> p t", p=P))
            # dinv = 1/sqrt(deg): sqrt on ACT, then DVE reciprocal
            nc.scalar.activation(deg_col[:], deg_col[:], AOT.Sqrt)
            nc.vector.reciprocal(dinv[:], deg_col[:])

            bufA = pp.tile([P, NW * h_dim], f32, tag="bufA")

            # === 3 GCN layers ===
            for layer in range(3):
                # ---- dense: zn = (h_in @ W) * dinv, tile by tile ----
                for t in range(NW):
                    if layer == 0:
                        xt = dp.tile([P, d_in], f32, tag="xt")
                        nc.sync.dma_start(xt[:], x_d[t * P:(t + 1) * P, :])
                        tp = ps1.tile([d_in, P], f32, tag="tps")
                        nc.tensor.transpose(tp[:], xt[:], ident[:])
                        sbT = dp.tile([d_in, P], f32, tag="sbT")
                        nc.vector.tensor_copy(sbT[:], tp[:])
                        kdim = d_in
                    else:
                        tp = ps1.tile([h_dim, P], f32, tag="tps")
                        nc.tensor.transpose(
                            tp[:], bufA[:, t * h_dim:(t + 1) * h_dim], ident[:])
                        sbT = dp.tile([h_dim, P], f32, tag="sbT")
                        nc.vector.tensor_copy(sbT[:], tp[:])
                        kdim = h_dim
                    zps = ps1.tile([P, h_dim], f32, tag="zps")
                    nc.tensor.matmul(zps[:], lhsT=sbT[:], rhs=W_sb[layer][:],
                                     start=True, stop=True)
                    nc.vector.tensor_scalar(
                        out=bufA[:, t * h_dim:(t + 1) * h_dim], in0=zps[:],
                        scalar1=dinv[:, t:t + 1], scalar2=None, op0=ALU.mult)

                # ---- publish zn + AllGather ----
                nc.sync.dma_start(
                    zn_d.rearrange("(t p) d -> p t d", p=P),
                    bufA[:].rearrange("p (t d) -> p t d", d=h_dim))
                nc.gpsimd.collective_compute(
                    "AllGather", ALU.bypass, replica_groups=rg,
                    ins=[zn_d[:]], outs=[table_d[:]])

                # ---- sparse aggregation: chunk-major gather + one-hot matmul ----
                wpsum = None
                for k in range(NCHUNK):
                    seg0, seg1 = chunk_off[k], chunk_off[k + 1]
                    for a in range(seg0, seg1, GRANULE):
                        gsz = min(GRANULE, seg1 - a)
                        gT = gsz // P
                        msg = msgp.tile([P, gT * h_dim], f32, tag="msg")
                        nc.gpsimd.dma_gather(
                            msg[:].rearrange("p (t d) -> p t d", d=h_dim),
                            chunk_views[k],
                            idx_sb[:, a // 16:(a + gsz) // 16],
                            gsz, gsz, h_dim, elem_step=NCHUNK * h_dim,
                            single_packet=False)
                        for i in range(gT):
                            t = a // P + i
                            w = int(tile_win[t])
                            oh = wp.tile([P, WIN], f32, tag="oh")
                            nc.vector.tensor_scalar(
                                out=oh[:], in0=iota_w[:],
                                scalar1=dstl_sb[:, t:t + 1], scalar2=None,
                                op0=ALU.is_equal)
                            if tile_first[t]:
                                wpsum = psp.tile([WIN, h_dim], f32, tag="wps")
                            nc.tensor.matmul(
                                wpsum[:], lhsT=oh[:],
                                rhs=msg[:, i * h_dim:(i + 1) * h_dim],
                                start=bool(tile_first[t]),
                                stop=bool(tile_last[t]))
                            if tile_last[t]:
                                dst = bufA[:, w * h_dim:(w + 1) * h_dim]
                                if k == 0:
                                    nc.vector.tensor_copy(dst, wpsum[:])
                                else:
                                    nc.vector.tensor_tensor(
                                        out=dst, in0=dst, in1=wpsum[:],
                                        op=ALU.add)

                # ---- flush: h = tanh(dinv * acc + b), in place ----
                for w in range(NW):
                    sl = bufA[:, w * h_dim:(w + 1) * h_dim]
                    tmp = wp.tile([P, h_dim], f32, tag="ftmp")
                    nc.vector.tensor_scalar(
                        out=tmp[:], in0=sl, scalar1=dinv[:, w:w + 1],
                        scalar2=None, op0=ALU.mult)
                    nc.vector.tensor_tensor(out=tmp[:], in0=tmp[:],
                                            in1=b_sb[layer][:], op=ALU.add)
                    nc.scalar.activation(sl, tmp[:], AOT.Tanh)

            # === pooling: pooledT[64, GMAX] = sum_h3 by graph ===
            poolT = poolps.tile([h_dim, GMAX], f32, tag="poolT")
            for t in range(NW):
                ohg = wp.tile([P, GMAX], f32, tag="ohg")
                nc.vector.tensor_scalar(
                    out=ohg[:], in0=iota_g[:], scalar1=batchl_sb[:, t:t + 1],
                    scalar2=None, op0=ALU.is_equal)
                nc.tensor.matmul(poolT[:],
                                 lhsT=bufA[:, t * h_dim:(t + 1) * h_dim],
                                 rhs=ohg[:], start=(t == 0), stop=(t == NW - 1))
            poolS = pp.tile([h_dim, GMAX], f32, tag="poolS")
            nc.vector.tensor_copy(poolS[:], poolT[:])

            # === head: out = tanh(pooled @ Wf + bf) ===
            for gt in range(GMAX // P):
                fps = psp.tile([P, n_cls], f32, tag="wps")
                nc.tensor.matmul(fps[:], lhsT=poolS[:, gt * P:(gt + 1) * P],
                                 rhs=Wf_sb[:], start=True, stop=True)
                ot = wp.tile([P, n_cls], f32, tag="ot")
                nc.vector.tensor_tensor(out=ot[:], in0=fps[:], in1=bf_sb[:],
                                        op=ALU.add)
                nc.scalar.activation(ot[:], ot[:], AOT.Tanh)
                nc.sync.dma_start(out_d[gt * P:(gt + 1) * P, :], ot[:])

    nc.compile()
    return nc


# ----------------------------------------------------------------------------
# Runner (persistent compiled program + per-core inputs)
# ----------------------------------------------------------------------------

class Runner:
    def __init__(self, meta, nc, d_in, h_dim, n_cls):
        self.meta = meta
        self.nc = nc
        self.d_in, self.h_dim, self.n_cls = d_in, h_dim, n_cls

    def in_maps(self, x, W1, b1, W2, b2, W3, b3, Wf, bf):
        m = self.meta
        S = m["S"]
        C = N_CORES
        x = np.asarray(x, np.float32)
        maps = []
        reps = dict(
            W1=np.asarray(W1, np.float32), W2=np.asarray(W2, np.float32),
            W3=np.asarray(W3, np.float32), Wf=np.asarray(Wf, np.float32),
            b1b=np.broadcast_to(np.asarray(b1, np.float32), (P, self.h_dim)).copy(),
            b2b=np.broadcast_to(np.asarray(b2, np.float32), (P, self.h_dim)).copy(),
            b3b=np.broadcast_to(np.asarray(b3, np.float32), (P, self.h_dim)).copy(),
            bfb=np.broadcast_to(np.asarray(bf, np.float32), (P, self.n_cls)).copy(),
            ident=np.eye(P, dtype=np.float32),
            iota_w=np.broadcast_to(np.arange(WIN, dtype=np.float32), (P, WIN)).copy(),
            iota_g=np.broadcast_to(np.arange(GMAX, dtype=np.float32), (P, GMAX)).copy(),
        )
        for c in range(C):
            n0, n1 = int(m["node_bnds"][c]), int(m["node_bnds"][c + 1])
            xl = np.zeros((S, self.d_in), np.float32)
            xl[: n1 - n0] = x[n0:n1]
            maps.append(dict(
                x_loc=xl,
                deg_loc=m["deg_pad"][c],
                idx16=np.tile(m["idx_wrapped"][c], (8, 1)),
                dstl=m["dstl_wrapped"][c],
                batchl=m["batchl_wrapped"][c],
                **reps,
            ))
        return maps

    def run(self, maps):
        from concourse.bass_utils import run_bass_kernel_spmd
        res = run_bass_kernel_spmd(self.nc, maps, list(range(N_CORES)))
        return self.assemble(res.results)

    def make_timed(self, maps):
        """Build a callable with inputs resident on device; each call runs the
        NEFF once and returns per-core outputs. For timing (transfer excluded)."""
        import jax
        import concourse.mybir as mybir
        from concourse import bass2jax
        from jax.experimental.shard_map import shard_map
        from jax.sharding import Mesh, NamedSharding, PartitionSpec

        nc = self.nc
        bass2jax.install_neuronx_cc_hook()
        partition_name = (nc.partition_id_tensor.name
                          if nc.partition_id_tensor else None)
        in_names, out_names, out_avals, zero_outs = [], [], [], []
        for alloc in nc.m.functions[0].allocations:
            if not isinstance(alloc, mybir.MemoryLocationSet):
                continue
            name = alloc.memorylocations[0].name
            if alloc.kind == "ExternalInput":
                if name != partition_name:
                    in_names.append(name)
            elif alloc.kind == "ExternalOutput":
                shape = tuple(alloc.tensor_shape)
                dtype = mybir.dt.np(alloc.dtype)
                out_names.append(name)
                out_avals.append(jax.core.ShapedArray(shape, dtype))
                zero_outs.append(np.zeros(shape, dtype))
        n_params = len(in_names)
        all_in = list(in_names) + list(out_names)
        if partition_name is not None:
            all_in.append(partition_name)
        donate = tuple(range(n_params, n_params + len(out_names)))

        def _body(*args):
            operands = list(args)
            if partition_name is not None:
                operands.append(bass2jax.partition_id_tensor())
            return tuple(bass2jax._bass_exec_p.bind(
                *operands, out_avals=tuple(out_avals), in_names=tuple(all_in),
                out_names=tuple(out_names), lowering_input_output_aliases=(),
                sim_require_finite=True, sim_require_nnan=True, nc=nc))

        devices = jax.devices()[:N_CORES]
        mesh = Mesh(np.asarray(devices), ("core",))
        spec = NamedSharding(mesh, PartitionSpec("core"))
        fn = jax.jit(shard_map(_body, mesh=mesh,
                               in_specs=(PartitionSpec("core"),) * (n_params + len(out_names)),
                               out_specs=(PartitionSpec("core"),) * len(out_names)),
                     donate_argnums=donate, keep_unused=True)
        dev_in = [jax.device_put(
            np.concatenate([np.asarray(maps[c][nm]) for c in range(N_CORES)], axis=0),
            spec) for nm in in_names]
        zshapes = [(N_CORES * z.shape[0], *z.shape[1:]) for z in zero_outs]
        zdtypes = [z.dtype for z in zero_outs]

        def call():
            zs = [jax.device_put(np.zeros(s, d), spec)
                  for s, d in zip(zshapes, zdtypes)]
            outs = fn(*dev_in, *zs)
            return [o.block_until_ready() for o in outs]

        return call, out_names, out_avals

    def assemble(self, results):
        m = self.meta
        outs = []
        for c in range(N_CORES):
            outs.append(results[c]["out"][: int(m["g_cnt"][c])])
        return np.concatenate(outs, axis=0)


_CACHE = {}


def _get_runner(edge_index, batch, n_nodes, n_graphs, d_in, h_dim, n_cls):
    key = (hashlib.sha1(np.ascontiguousarray(edge_index).tobytes()).hexdigest(),
           hashlib.sha1(np.ascontiguousarray(batch).tobytes()).hexdigest(),
           n_nodes, n_graphs, d_in, h_dim, n_cls)
    r = _CACHE.get(key)
    if r is None:
        meta = _preprocess(edge_index, batch, n_nodes, n_graphs)
        nc = _build_program(meta, d_in, h_dim, n_cls)
        r = Runner(meta, nc, d_in, h_dim, n_cls)
        _CACHE[key] = r
    return r


def kernel(x, edge_index, batch, W1, b1, W2, b2, W3, b3, Wf, bf):
    x = np.asarray(x)
    r = _get_runner(np.asarray(edge_index), np.asarray(batch), x.shape[0],
                    N_GRAPHS, x.shape[1], np.asarray(W1).shape[1],
                    np.asarray(Wf).shape[1])
    maps = r.in_maps(x, W1, b1, W2, b2, W3, b3, Wf, bf)
    return r.run(maps)



# revision 21
# speedup vs baseline: 820.1580x; 820.1580x over previous
"""GCN (3-layer GCNConv + global_add_pool + linear head) on 8 Trainium2 cores.

Strategy:
 - Nodes sharded across 8 cores on graph-id-aligned boundaries (pooling local).
 - Edges partitioned by dst owner. Per core, edges ordered chunk-major
   (src_row % 4 -> int16 gather index fits), then by 128-node dst window,
   padded per (chunk, window) group to multiples of 128 and uniform tile
   counts across cores (single SPMD program).
 - Per layer: dense z = h @ W on PE (transpose-on-the-fly, 8 z-tiles batched
   per PSUM bank), one batched zn = z * dinv, AllGather zn -> full table in
   DRAM, dma_gather 256B rows per edge round-robined over the 4 SWDGE queues
   (concurrent Q7 descriptor-gen + deeper DMA pipelining), per-granule
   batched one-hot generation (single broadcast-AP is_equal -> bf16),
   ACT-engine msg cast f32->bf16, segment-sum via bf16 one-hot matmuls
   accumulated into 8-window PSUM banks, per-bank flush into SBUF across the
   4 chunk passes, then one batched tanh(dinv*acc + b) per layer.
 - Pooling: batched one-hot(graph id) generation + matmul into a [64, 512]
   PSUM tile; final linear head + tanh on device.
 - Instruction-count economics are the main lever: one-hot gen, scaling,
   bias, tanh and psum flushes are all batched (engine sequencers otherwise
   serialize at ~0.3-0.9us per instruction).
"""

import hashlib
import sys

for _p in ("/opt/trn_rl_repo",):
    if _p not in sys.path:
        sys.path.insert(0, _p)

import numpy as np

def _bfnp():
    import concourse.mybir as mybir
    return mybir.dt.np(mybir.dt.bfloat16)

P = 128
WIN = 128          # dst-window width (nodes)
NCHUNK = 4         # src chunks (int16 index limit: 8S/4 <= 32767)
GRANULE = 4096     # idxs per dma_gather instruction
N_CORES = 8
N_GRAPHS = 2048    # problem constant
GMAX = 512         # per-core graph-count upper bound (psum free dim)
BLK = 8            # windows per psum accumulation bank (512 f32)


# ----------------------------------------------------------------------------
# Host-side sharding / edge bucketing (index manipulation only, no float math)
# ----------------------------------------------------------------------------

def _preprocess(edge_index, batch, n_nodes, n_graphs):
    C = N_CORES
    src = np.asarray(edge_index[0], dtype=np.int64)
    dst = np.asarray(edge_index[1], dtype=np.int64)
    batch = np.asarray(batch, dtype=np.int64)
    N = n_nodes

    # graph-aligned node shard boundaries
    gstart = np.searchsorted(batch, np.arange(n_graphs + 1))  # [G+1], gstart[G] = N
    node_bnds = [0]
    g_bnds = [0]
    for c in range(1, C):
        tgt = (c * N) // C
        g = int(np.searchsorted(gstart, tgt))
        # candidates g-1, g: pick nearest boundary node
        if g > 0 and abs(int(gstart[g - 1]) - tgt) <= abs(int(gstart[min(g, n_graphs)]) - tgt):
            g = g - 1
        g = min(max(g, g_bnds[-1]), n_graphs)
        g_bnds.append(g)
        node_bnds.append(int(gstart[g]))
    node_bnds.append(N)
    g_bnds.append(n_graphs)
    node_bnds = np.array(node_bnds, dtype=np.int64)          # [C+1]
    g_bnds = np.array(g_bnds, dtype=np.int64)                # [C+1]
    node_cnt = node_bnds[1:] - node_bnds[:-1]
    g_cnt = g_bnds[1:] - g_bnds[:-1]
    assert g_cnt.max() < GMAX - 1, g_cnt

    S = int(-(-node_cnt.max() // P) * P)                     # padded shard size
    NW = S // WIN                                            # windows per core
    assert 2 * S <= 32767, S                                 # int16 gather idx bound

    owner = np.searchsorted(node_bnds[1:], np.arange(N), side="right")
    local = np.arange(N) - node_bnds[owner]
    row = owner * S + local                                  # table row per node

    deg = np.bincount(dst, minlength=N).astype(np.float32) + 1.0

    # edge stream (+ self loops)
    e_src = np.concatenate([src, np.arange(N)])
    e_dst = np.concatenate([dst, np.arange(N)])
    e_owner = owner[e_dst]
    e_dl = local[e_dst]
    e_row = row[e_src]
    e_chunk = (e_row & (NCHUNK - 1)).astype(np.int64)
    e_idx = (e_row >> 2).astype(np.int16)
    e_win = e_dl >> 7

    key = (e_chunk * NW + e_win) * C + e_owner               # chunk-major, then window
    order = np.argsort(key, kind="stable")
    cnt = np.bincount(key, minlength=NCHUNK * NW * C).reshape(NCHUNK, NW, C)

    tiles_kw = -(-cnt.max(axis=2) // P)                      # [NCHUNK, NW] uniform tiles
    # every (chunk, window) needs >=1 tile so the 8-window psum blocks are
    # fully written before each block flush
    tiles_kw = np.maximum(tiles_kw, 1)
    pad_kw = tiles_kw * P                                    # padded group sizes
    E_PAD = int(pad_kw.sum())
    # group start offsets in the uniform stream (same for all cores)
    goff = np.zeros((NCHUNK, NW), dtype=np.int64)
    goff.flat[1:] = np.cumsum(pad_kw.flat)[:-1]

    idx16 = np.zeros((C, E_PAD), dtype=np.int16)             # pad -> idx 0 (valid row)
    dstl = np.full((C, E_PAD), -1.0, dtype=np.float32)       # pad -> -1 (one-hot miss)

    # place real edges
    so = order
    r_owner = e_owner[so]
    r_chunk = e_chunk[so]
    r_win = e_win[so]
    # position within (chunk, win, owner) group = running index
    rkey = (r_chunk * NW + r_win) * C + r_owner
    # stable sort => positions are 0..cnt-1 in order of appearance
    pos = np.zeros(len(so), dtype=np.int64)
    _, first_idx, inv = np.unique(rkey, return_index=True, return_inverse=True)
    pos = np.arange(len(so)) - first_idx[inv]
    slot = goff[r_chunk, r_win] + pos
    idx16[r_owner, slot] = e_idx[so]
    dstl[r_owner, slot] = (e_dl[so] - r_win * WIN).astype(np.float32)

    # per-tile metadata (uniform): window id, group-first, group-last,
    # 8-window psum-block first/last
    TILES = E_PAD // P
    tile_win = np.zeros(TILES, dtype=np.int64)
    tile_first = np.zeros(TILES, dtype=bool)
    tile_last = np.zeros(TILES, dtype=bool)
    blk_first = np.zeros(TILES, dtype=bool)
    blk_last = np.zeros(TILES, dtype=bool)
    for k in range(NCHUNK):
        for w in range(NW):
            t0 = goff[k, w] // P
            nt = int(tiles_kw[k, w])
            assert nt > 0
            tile_win[t0:t0 + nt] = w
            tile_first[t0] = True
            tile_last[t0 + nt - 1] = True
            if w % BLK == 0:
                blk_first[t0] = True
            if w % BLK == BLK - 1 or w == NW - 1:
                blk_last[t0 + nt - 1] = True
    # chunk segment boundaries (in idx positions)
    chunk_off = [int(goff[k, 0]) for k in range(NCHUNK)] + [E_PAD]

    # gather-layout idx: [16, E_PAD/16] with [p, s] = stream[s*16+p]
    idx_wrapped = np.ascontiguousarray(
        idx16.reshape(C, E_PAD // 16, 16).transpose(0, 2, 1))
    # dstl layout [128, E_PAD/128] with [p, t] = stream[t*128+p]
    dstl_wrapped = np.ascontiguousarray(
        dstl.reshape(C, TILES, P).transpose(0, 2, 1))

    # per-core padded node arrays
    deg_pad = np.ones((C, S), dtype=np.float32)
    batchl = np.full((C, S), float(GMAX - 1), dtype=np.float32)
    for c in range(C):
        n0, n1 = int(node_bnds[c]), int(node_bnds[c + 1])
        deg_pad[c, : n1 - n0] = deg[n0:n1]
        batchl[c, : n1 - n0] = (batch[n0:n1] - g_bnds[c]).astype(np.float32)
    batchl_wrapped = np.ascontiguousarray(
        batchl.reshape(C, NW, P).transpose(0, 2, 1))         # [C, 128, NW]

    return dict(
        S=S, NW=NW, E_PAD=E_PAD, TILES=TILES,
        node_bnds=node_bnds, g_bnds=g_bnds, node_cnt=node_cnt, g_cnt=g_cnt,
        idx_wrapped=idx_wrapped, dstl_wrapped=dstl_wrapped,
        batchl_wrapped=batchl_wrapped, deg_pad=deg_pad,
        tile_win=tile_win, tile_first=tile_first, tile_last=tile_last,
        blk_first=blk_first, blk_last=blk_last,
        chunk_off=chunk_off,
    )


# ----------------------------------------------------------------------------
# Bass program builder
# ----------------------------------------------------------------------------

def _build_program(meta, d_in, h_dim, n_cls):
    import concourse.bacc as bacc
    import concourse.mybir as mybir
    import concourse.tile as tile
    from concourse import library_config

    S, NW, E_PAD = meta["S"], meta["NW"], meta["E_PAD"]
    tile_win = meta["tile_win"]
    tile_first = meta["tile_first"]
    tile_last = meta["tile_last"]
    blk_first = meta["blk_first"]
    blk_last = meta["blk_last"]
    chunk_off = meta["chunk_off"]
    f32 = mybir.dt.float32
    bf16 = mybir.dt.bfloat16
    AOT = mybir.ActivationFunctionType
    ALU = mybir.AluOpType

    nc = bacc.Bacc("TRN2", target_bir_lowering=False, debug=False,
                   num_devices=N_CORES, num_swdge_queues=4)

    # --- I/O ---
    x_d = nc.dram_tensor("x_loc", [S, d_in], f32, kind="ExternalInput").ap()
    deg_d = nc.dram_tensor("deg_loc", [S], f32, kind="ExternalInput").ap()
    idx_d = nc.dram_tensor("idx16", [P, E_PAD // 16], mybir.dt.int16,
                           kind="ExternalInput").ap()
    dstl_d = nc.dram_tensor("dstl", [P, E_PAD // P], bf16,
                            kind="ExternalInput").ap()
    batchl_d = nc.dram_tensor("batchl", [P, NW], f32, kind="ExternalInput").ap()
    W_d = [nc.dram_tensor("W1", [d_in, h_dim], f32, kind="ExternalInput").ap(),
           nc.dram_tensor("W2", [h_dim, h_dim], f32, kind="ExternalInput").ap(),
           nc.dram_tensor("W3", [h_dim, h_dim], f32, kind="ExternalInput").ap()]
    Wf_d = nc.dram_tensor("Wf", [h_dim, n_cls], f32, kind="ExternalInput").ap()
    b_d = [nc.dram_tensor(f"b{i+1}b", [P, h_dim], f32, kind="ExternalInput").ap()
           for i in range(3)]
    bf_d = nc.dram_tensor("bfb", [P, n_cls], f32, kind="ExternalInput").ap()
    out_d = nc.dram_tensor("out", [GMAX, n_cls], f32, kind="ExternalOutput").ap()
    ident_d = nc.dram_tensor("ident", [P, P], f32, kind="ExternalInput").ap()
    iota_w_d = nc.dram_tensor("iota_w", [P, WIN], bf16, kind="ExternalInput").ap()
    iota_g_d = nc.dram_tensor("iota_g", [P, GMAX], f32, kind="ExternalInput").ap()

    zn_d = nc.dram_tensor("zn_loc", [S, h_dim], f32).ap()
    table_d = nc.dram_tensor("table", [N_CORES * S, h_dim], f32,
                             addr_space="Shared").ap()
    chunk_views = table_d.rearrange("(n four) d -> four n d", four=NCHUNK)
    rg = [list(range(N_CORES))]

    with tile.TileContext(nc) as tc:
        with (
            tc.tile_pool(name="persist", bufs=1) as pp,
            tc.tile_pool(name="msg", bufs=8) as msgp,
            tc.tile_pool(name="ohp", bufs=2) as ohp,
            tc.tile_pool(name="work", bufs=4) as wp,
            tc.tile_pool(name="dense", bufs=3) as dp,
            tc.tile_pool(name="psum", bufs=2, space="PSUM") as psp,
            tc.tile_pool(name="psum1", bufs=2, space="PSUM") as ps1,
            tc.tile_pool(name="pool_ps", bufs=1, space="PSUM") as poolps,
        ):
            # --- persistent tiles ---
            nc.gpsimd.load_library(library_config.mlp)
            ident = pp.tile([P, P], f32, tag="ident")
            nc.sync.dma_start(ident[:], ident_d[:])
            iota_w = pp.tile([P, WIN], bf16, tag="iota_w")
            nc.sync.dma_start(iota_w[:], iota_w_d[:])
            iota_g = pp.tile([P, GMAX], f32, tag="iota_g")
            nc.sync.dma_start(iota_g[:], iota_g_d[:])

            W_sb = []
            for i in range(3):
                k = d_in if i == 0 else h_dim
                t = pp.tile([k, h_dim], f32, tag=f"W{i}")
                nc.sync.dma_start(t[:], W_d[i][:])
                W_sb.append(t)
            Wf_sb = pp.tile([h_dim, n_cls], f32, tag="Wf")
            nc.sync.dma_start(Wf_sb[:], Wf_d[:])
            b_sb = []
            for i in range(3):
                t = pp.tile([P, h_dim], f32, tag=f"b{i}")
                nc.sync.dma_start(t[:], b_d[i][:])
                b_sb.append(t)
            bf_sb = pp.tile([P, n_cls], f32, tag="bf")
            nc.sync.dma_start(bf_sb[:], bf_d[:])

            idx_sb = pp.tile([P, E_PAD // 16], mybir.dt.int16, tag="idx")
            nc.sync.dma_start(idx_sb[:], idx_d[:])
            dstl_sb = pp.tile([P, E_PAD // P], bf16, tag="dstl")
            nc.sync.dma_start(dstl_sb[:], dstl_d[:])
            batchl_sb = pp.tile([P, NW], f32, tag="batchl")
            nc.sync.dma_start(batchl_sb[:], batchl_d[:])

            dinv = pp.tile([P, NW], f32, tag="dinv")
            deg_col = pp.tile([P, NW], f32, tag="degc")
            nc.sync.dma_start(deg_col[:], deg_d.rearrange("(t p) -> p t", p=P))
            # dinv = 1/sqrt(deg): sqrt on ACT, then DVE reciprocal
            nc.scalar.activation(deg_col[:], deg_col[:], AOT.Sqrt)
            nc.vector.reciprocal(dinv[:], deg_col[:])

            bufA = pp.tile([P, NW * h_dim], f32, tag="bufA")

            bufA3 = bufA[:].rearrange("p (w d) -> p w d", d=h_dim)
            dinv_bc = dinv[:].unsqueeze(2).broadcast_to([P, NW, h_dim])

            # === 3 GCN layers ===
            for layer in range(3):
                # ---- dense: z = h_in @ W, 8 tiles per psum bank ----
                zbank = None
                for t in range(NW):
                    if layer == 0:
                        xt = dp.tile([P, d_in], f32, tag="xt")
                        nc.sync.dma_start(xt[:], x_d[t * P:(t + 1) * P, :])
                        tp = ps1.tile([d_in, P], f32, tag="tps")
                        nc.tensor.transpose(tp[:], xt[:], ident[:])
                        sbT = dp.tile([d_in, P], f32, tag="sbT")
                        nc.vector.tensor_copy(sbT[:], tp[:])
                    else:
                        tp = ps1.tile([h_dim, P], f32, tag="tps")
                        nc.tensor.transpose(
                            tp[:], bufA[:, t * h_dim:(t + 1) * h_dim], ident[:])
                        sbT = dp.tile([h_dim, P], f32, tag="sbT")
                        nc.vector.tensor_copy(sbT[:], tp[:])
                    if t % BLK == 0:
                        zbank = psp.tile([P, BLK * h_dim], f32, tag="zbank")
                    zsl = zbank[:, (t % BLK) * h_dim:(t % BLK + 1) * h_dim]
                    nc.tensor.matmul(zsl, lhsT=sbT[:], rhs=W_sb[layer][:],
                                     start=True, stop=True)
                    if t % BLK == BLK - 1 or t == NW - 1:
                        t0 = (t // BLK) * BLK
                        nc.vector.tensor_copy(
                            bufA[:, t0 * h_dim:(t + 1) * h_dim],
                            zbank[:, :(t - t0 + 1) * h_dim])
                # zn = z * dinv, one batched instr
                nc.vector.tensor_tensor(out=bufA3, in0=bufA3, in1=dinv_bc,
                                        op=ALU.mult)

                # ---- publish zn + AllGather ----
                nc.sync.dma_start(
                    zn_d.rearrange("(t p) d -> p t d", p=P),
                    bufA[:].rearrange("p (t d) -> p t d", d=h_dim))
                nc.gpsimd.collective_compute(
                    "AllGather", ALU.bypass, replica_groups=rg,
                    ins=[zn_d[:]], outs=[table_d[:]])

                # ---- sparse aggregation: 4-queue gathers + batched one-hot
                #      gen + psum-block matmul accumulation ----
                wpsum = None
                qcnt = 0
                for k in range(NCHUNK):
                    seg0, seg1 = chunk_off[k], chunk_off[k + 1]
                    for a in range(seg0, seg1, GRANULE):
                        gsz = min(GRANULE, seg1 - a)
                        gT = gsz // P
                        msg = msgp.tile([P, gT * h_dim], f32, tag="msg")
                        nc.gpsimd.dma_gather(
                            msg[:].rearrange("p (t d) -> p t d", d=h_dim),
                            chunk_views[k],
                            idx_sb[:, a // 16:(a + gsz) // 16],
                            gsz, gsz, h_dim, elem_step=NCHUNK * h_dim,
                            single_packet=False, queue_num=qcnt % 4)
                        qcnt += 1
                        t0g = a // P
                        msg16 = msgp.tile([P, gT * h_dim], bf16, tag="msg16")
                        nc.scalar.copy(msg16[:], msg[:])
                        ohblk = ohp.tile([P, gT * WIN], bf16, tag="ohblk")
                        nc.vector.tensor_tensor(
                            out=ohblk[:].rearrange("p (t w) -> p t w", w=WIN),
                            in0=iota_w[:].unsqueeze(1).broadcast_to(
                                [P, gT, WIN]),
                            in1=dstl_sb[:, t0g:t0g + gT].unsqueeze(2)
                                .broadcast_to([P, gT, WIN]),
                            op=ALU.is_equal)
                        for i in range(gT):
                            t = t0g + i
                            w = int(tile_win[t])
                            if blk_first[t]:
                                wpsum = psp.tile([P, BLK * h_dim], f32,
                                                 tag="wps")
                            nc.tensor.matmul(
                                wpsum[:, (w % BLK) * h_dim:
                                      (w % BLK + 1) * h_dim],
                                lhsT=ohblk[:, i * WIN:(i + 1) * WIN],
                                rhs=msg16[:, i * h_dim:(i + 1) * h_dim],
                                start=bool(tile_first[t]),
                                stop=bool(tile_last[t]))
                            if blk_last[t]:
                                w0 = (w // BLK) * BLK
                                dst = bufA[:, w0 * h_dim:(w + 1) * h_dim]
                                src = wpsum[:, :(w - w0 + 1) * h_dim]
                                if k == 0:
                                    nc.vector.tensor_copy(dst, src)
                                else:
                                    nc.vector.tensor_tensor(
                                        out=dst, in0=dst, in1=src, op=ALU.add)

                # ---- flush: h = tanh(dinv * acc + b), batched in place ----
                nc.vector.tensor_tensor(out=bufA3, in0=bufA3, in1=dinv_bc,
                                        op=ALU.mult)
                nc.vector.tensor_tensor(
                    out=bufA3, in0=bufA3,
                    in1=b_sb[layer][:].unsqueeze(1).broadcast_to(
                        [P, NW, h_dim]),
                    op=ALU.add)
                nc.scalar.activation(bufA[:], bufA[:], AOT.Tanh)

            # === pooling: pooledT[64, GMAX] = sum_h3 by graph ===
            poolT = poolps.tile([h_dim, GMAX], f32, tag="poolT")
            NBW = -(-NW // BLK)
            ohg_blk = pp.tile([P, BLK * GMAX], f32, tag="ohgblk")
            for bw in range(NBW):
                w0 = bw * BLK
                nw = min(BLK, NW - w0)
                nc.vector.tensor_tensor(
                    out=ohg_blk[:, :nw * GMAX].rearrange(
                        "p (t g) -> p t g", g=GMAX),
                    in0=iota_g[:].unsqueeze(1).broadcast_to([P, nw, GMAX]),
                    in1=batchl_sb[:, w0:w0 + nw].unsqueeze(2)
                        .broadcast_to([P, nw, GMAX]),
                    op=ALU.is_equal)
                for i in range(nw):
                    t = w0 + i
                    nc.tensor.matmul(
                        poolT[:],
                        lhsT=bufA[:, t * h_dim:(t + 1) * h_dim],
                        rhs=ohg_blk[:, i * GMAX:(i + 1) * GMAX],
                        start=(t == 0), stop=(t == NW - 1))
            poolS = pp.tile([h_dim, GMAX], f32, tag="poolS")
            nc.vector.tensor_copy(poolS[:], poolT[:])

            # === head: out = tanh(pooled @ Wf + bf) ===
            for gt in range(GMAX // P):
                fps = psp.tile([P, n_cls], f32, tag="wps")
                nc.tensor.matmul(fps[:], lhsT=poolS[:, gt * P:(gt + 1) * P],
                                 rhs=Wf_sb[:], start=True, stop=True)
                ot = wp.tile([P, n_cls], f32, tag="ot")
                nc.vector.tensor_tensor(out=ot[:], in0=fps[:], in1=bf_sb[:],
                                        op=ALU.add)
                nc.scalar.activation(ot[:], ot[:], AOT.Tanh)
                nc.sync.dma_start(out_d[gt * P:(gt + 1) * P, :], ot[:])

    nc.compile()
    return nc


# ----------------------------------------------------------------------------
# Runner (persistent compiled program + per-core inputs)
# ----------------------------------------------------------------------------

class Runner:
    def __init__(self, meta, nc, d_in, h_dim, n_cls):
        self.meta = meta
        self.nc = nc
        self.d_in, self.h_dim, self.n_cls = d_in, h_dim, n_cls

    def in_maps(self, x, W1, b1, W2, b2, W3, b3, Wf, bf):
        _bf = _bfnp()
        m = self.meta
        S = m["S"]
        C = N_CORES
        x = np.asarray(x, np.float32)
        maps = []
        reps = dict(
            W1=np.asarray(W1, np.float32), W2=np.asarray(W2, np.float32),
            W3=np.asarray(W3, np.float32), Wf=np.asarray(Wf, np.float32),
            b1b=np.broadcast_to(np.asarray(b1, np.float32), (P, self.h_dim)).copy(),
            b2b=np.broadcast_to(np.asarray(b2, np.float32), (P, self.h_dim)).copy(),
            b3b=np.broadcast_to(np.asarray(b3, np.float32), (P, self.h_dim)).copy(),
            bfb=np.broadcast_to(np.asarray(bf, np.float32), (P, self.n_cls)).copy(),
            ident=np.eye(P, dtype=np.float32),
            iota_w=np.broadcast_to(np.arange(WIN, dtype=np.float32), (P, WIN)).astype(_bf), 
            iota_g=np.broadcast_to(np.arange(GMAX, dtype=np.float32), (P, GMAX)).copy(),
        )
        for c in range(C):
            n0, n1 = int(m["node_bnds"][c]), int(m["node_bnds"][c + 1])
            xl = np.zeros((S, self.d_in), np.float32)
            xl[: n1 - n0] = x[n0:n1]
            maps.append(dict(
                x_loc=xl,
                deg_loc=m["deg_pad"][c],
                idx16=np.tile(m["idx_wrapped"][c], (8, 1)),
                dstl=m["dstl_wrapped"][c].astype(_bf),
                batchl=m["batchl_wrapped"][c],
                **reps,
            ))
        return maps

    def run(self, maps):
        from concourse.bass_utils import run_bass_kernel_spmd
        res = run_bass_kernel_spmd(self.nc, maps, list(range(N_CORES)))
        return self.assemble(res.results)

    def make_timed(self, maps):
        """Build a callable with inputs resident on device; each call runs the
        NEFF once and returns per-core outputs. For timing (transfer excluded)."""
        import jax
        import concourse.mybir as mybir
        from concourse import bass2jax
        from jax.experimental.shard_map import shard_map
        from jax.sharding import Mesh, NamedSharding, PartitionSpec

        nc = self.nc
        bass2jax.install_neuronx_cc_hook()
        partition_name = (nc.partition_id_tensor.name
                          if nc.partition_id_tensor else None)
        in_names, out_names, out_avals, zero_outs = [], [], [], []
        for alloc in nc.m.functions[0].allocations:
            if not isinstance(alloc, mybir.MemoryLocationSet):
                continue
            name = alloc.memorylocations[0].name
            if alloc.kind == "ExternalInput":
                if name != partition_name:
                    in_names.append(name)
            elif alloc.kind == "ExternalOutput":
                shape = tuple(alloc.tensor_shape)
                dtype = mybir.dt.np(alloc.dtype)
                out_names.append(name)
                out_avals.append(jax.core.ShapedArray(shape, dtype))
                zero_outs.append(np.zeros(shape, dtype))
        n_params = len(in_names)
        all_in = list(in_names) + list(out_names)
        if partition_name is not None:
            all_in.append(partition_name)
        donate = tuple(range(n_params, n_params + len(out_names)))

        def _body(*args):
            operands = list(args)
            if partition_name is not None:
                operands.append(bass2jax.partition_id_tensor())
            return tuple(bass2jax._bass_exec_p.bind(
                *operands, out_avals=tuple(out_avals), in_names=tuple(all_in),
                out_names=tuple(out_names), lowering_input_output_aliases=(),
                sim_require_finite=True, sim_require_nnan=True, nc=nc))

        devices = jax.devices()[:N_CORES]
        mesh = Mesh(np.asarray(devices), ("core",))
        spec = NamedSharding(mesh, PartitionSpec("core"))
        fn = jax.jit(shard_map(_body, mesh=mesh,
                               in_specs=(PartitionSpec("core"),) * (n_params + len(out_names)),
                               out_specs=(PartitionSpec("core"),) * len(out_names)),
                     donate_argnums=donate, keep_unused=True)
        dev_in = [jax.device_put(
            np.concatenate([np.asarray(maps[c][nm]) for c in range(N_CORES)], axis=0),
            spec) for nm in in_names]
        zshapes = [(N_CORES * z.shape[0], *z.shape[1:]) for z in zero_outs]
        zdtypes = [z.dtype for z in zero_outs]

        def call():
            zs = [jax.device_put(np.zeros(s, d), spec)
                  for s, d in zip(zshapes, zdtypes)]
            outs = fn(*dev_in, *zs)
            return [o.block_until_ready() for o in outs]

        return call, out_names, out_avals

    def assemble(self, results):
        m = self.meta
        outs = []
        for c in range(N_CORES):
            outs.append(results[c]["out"][: int(m["g_cnt"][c])])
        return np.concatenate(outs, axis=0)


_CACHE = {}


def _get_runner(edge_index, batch, n_nodes, n_graphs, d_in, h_dim, n_cls):
    key = (hashlib.sha1(np.ascontiguousarray(edge_index).tobytes()).hexdigest(),
           hashlib.sha1(np.ascontiguousarray(batch).tobytes()).hexdigest(),
           n_nodes, n_graphs, d_in, h_dim, n_cls)
    r = _CACHE.get(key)
    if r is None:
        meta = _preprocess(edge_index, batch, n_nodes, n_graphs)
        nc = _build_program(meta, d_in, h_dim, n_cls)
        r = Runner(meta, nc, d_in, h_dim, n_cls)
        _CACHE[key] = r
    return r


def kernel(x, edge_index, batch, W1, b1, W2, b2, W3, b3, Wf, bf):
    x = np.asarray(x)
    r = _get_runner(np.asarray(edge_index), np.asarray(batch), x.shape[0],
                    N_GRAPHS, x.shape[1], np.asarray(W1).shape[1],
                    np.asarray(Wf).shape[1])
    maps = r.in_maps(x, W1, b1, W2, b2, W3, b3, Wf, bf)
    return r.run(maps)



# revision 22
# speedup vs baseline: 838.8701x; 1.0228x over previous
"""GCN (3-layer GCNConv + global_add_pool + linear head) on 8 Trainium2 cores.

Strategy:
 - Nodes sharded across 8 cores on graph-id-aligned boundaries (pooling local).
 - Edges partitioned by dst owner. Per core, edges ordered chunk-major
   (src_row % 4 -> int16 gather index fits), then by 128-node dst window,
   padded per (chunk, window) group to multiples of 128 and uniform tile
   counts across cores (single SPMD program).
 - Per layer: dense z = h @ W on PE (transpose-on-the-fly, 8 z-tiles batched
   per PSUM bank), one batched zn = z * dinv, AllGather zn -> full table in
   DRAM, dma_gather 256B rows per edge round-robined over the 4 SWDGE queues
   (concurrent Q7 descriptor-gen + deeper DMA pipelining), per-granule
   batched one-hot generation (single broadcast-AP is_equal -> bf16),
   ACT-engine msg cast f32->bf16, segment-sum via bf16 one-hot matmuls
   accumulated into 8-window PSUM banks, per-bank flush into SBUF across the
   4 chunk passes, then one batched tanh(dinv*acc + b) per layer.
 - Pooling: batched one-hot(graph id) generation + matmul into a [64, 512]
   PSUM tile; final linear head + tanh on device.
 - Instruction-count economics are the main lever: one-hot gen, scaling,
   bias, tanh and psum flushes are all batched (engine sequencers otherwise
   serialize at ~0.3-0.9us per instruction).
"""

import hashlib
import sys

for _p in ("/opt/trn_rl_repo",):
    if _p not in sys.path:
        sys.path.insert(0, _p)

import numpy as np

def _bfnp():
    import concourse.mybir as mybir
    return mybir.dt.np(mybir.dt.bfloat16)

P = 128
WIN = 128          # dst-window width (nodes)
NCHUNK = 4         # src chunks (int16 index limit: 8S/4 <= 32767)
GRANULE = 4096     # idxs per dma_gather instruction
N_CORES = 8
N_GRAPHS = 2048    # problem constant
GMAX = 512         # per-core graph-count upper bound (psum free dim)
BLK = 8            # windows per psum accumulation bank (512 f32)


# ----------------------------------------------------------------------------
# Host-side sharding / edge bucketing (index manipulation only, no float math)
# ----------------------------------------------------------------------------

def _preprocess(edge_index, batch, n_nodes, n_graphs):
    C = N_CORES
    src = np.asarray(edge_index[0], dtype=np.int64)
    dst = np.asarray(edge_index[1], dtype=np.int64)
    batch = np.asarray(batch, dtype=np.int64)
    N = n_nodes

    # graph-aligned node shard boundaries
    gstart = np.searchsorted(batch, np.arange(n_graphs + 1))  # [G+1], gstart[G] = N
    node_bnds = [0]
    g_bnds = [0]
    for c in range(1, C):
        tgt = (c * N) // C
        g = int(np.searchsorted(gstart, tgt))
        # candidates g-1, g: pick nearest boundary node
        if g > 0 and abs(int(gstart[g - 1]) - tgt) <= abs(int(gstart[min(g, n_graphs)]) - tgt):
            g = g - 1
        g = min(max(g, g_bnds[-1]), n_graphs)
        g_bnds.append(g)
        node_bnds.append(int(gstart[g]))
    node_bnds.append(N)
    g_bnds.append(n_graphs)
    node_bnds = np.array(node_bnds, dtype=np.int64)          # [C+1]
    g_bnds = np.array(g_bnds, dtype=np.int64)                # [C+1]
    node_cnt = node_bnds[1:] - node_bnds[:-1]
    g_cnt = g_bnds[1:] - g_bnds[:-1]
    assert g_cnt.max() < GMAX - 1, g_cnt

    S = int(-(-node_cnt.max() // P) * P)                     # padded shard size
    NW = S // WIN                                            # windows per core
    assert 2 * S <= 32767, S                                 # int16 gather idx bound

    owner = np.searchsorted(node_bnds[1:], np.arange(N), side="right")
    local = np.arange(N) - node_bnds[owner]
    row = owner * S + local                                  # table row per node

    deg = np.bincount(dst, minlength=N).astype(np.float32) + 1.0

    # edge stream (+ self loops)
    e_src = np.concatenate([src, np.arange(N)])
    e_dst = np.concatenate([dst, np.arange(N)])
    e_owner = owner[e_dst]
    e_dl = local[e_dst]
    S2 = S // 2
    e_loc = local[e_src]
    e_osrc = owner[e_src]
    e_half = (e_loc >= S2).astype(np.int64)
    rh = e_osrc * S2 + (e_loc - e_half * S2)     # row within half-table
    e_chunk = (2 * e_half + (rh & 1)).astype(np.int64)
    e_idx = (rh >> 1).astype(np.int16)           # < 4*S2 = 25088
    e_win = e_dl >> 7

    key = (e_chunk * NW + e_win) * C + e_owner               # chunk-major, then window
    order = np.argsort(key, kind="stable")
    cnt = np.bincount(key, minlength=NCHUNK * NW * C).reshape(NCHUNK, NW, C)

    tiles_kw = -(-cnt.max(axis=2) // P)                      # [NCHUNK, NW] uniform tiles
    # every (chunk, window) needs >=1 tile so the 8-window psum blocks are
    # fully written before each block flush
    tiles_kw = np.maximum(tiles_kw, 1)
    pad_kw = tiles_kw * P                                    # padded group sizes
    E_PAD = int(pad_kw.sum())
    # group start offsets in the uniform stream (same for all cores)
    goff = np.zeros((NCHUNK, NW), dtype=np.int64)
    goff.flat[1:] = np.cumsum(pad_kw.flat)[:-1]

    idx16 = np.zeros((C, E_PAD), dtype=np.int16)             # pad -> idx 0 (valid row)
    dstl = np.full((C, E_PAD), -1.0, dtype=np.float32)       # pad -> -1 (one-hot miss)

    # place real edges
    so = order
    r_owner = e_owner[so]
    r_chunk = e_chunk[so]
    r_win = e_win[so]
    # position within (chunk, win, owner) group = running index
    rkey = (r_chunk * NW + r_win) * C + r_owner
    # stable sort => positions are 0..cnt-1 in order of appearance
    pos = np.zeros(len(so), dtype=np.int64)
    _, first_idx, inv = np.unique(rkey, return_index=True, return_inverse=True)
    pos = np.arange(len(so)) - first_idx[inv]
    slot = goff[r_chunk, r_win] + pos
    idx16[r_owner, slot] = e_idx[so]
    dstl[r_owner, slot] = (e_dl[so] - r_win * WIN).astype(np.float32)

    # per-tile metadata (uniform): window id, group-first, group-last,
    # 8-window psum-block first/last
    TILES = E_PAD // P
    tile_win = np.zeros(TILES, dtype=np.int64)
    tile_first = np.zeros(TILES, dtype=bool)
    tile_last = np.zeros(TILES, dtype=bool)
    blk_first = np.zeros(TILES, dtype=bool)
    blk_last = np.zeros(TILES, dtype=bool)
    for k in range(NCHUNK):
        for w in range(NW):
            t0 = goff[k, w] // P
            nt = int(tiles_kw[k, w])
            assert nt > 0
            tile_win[t0:t0 + nt] = w
            tile_first[t0] = True
            tile_last[t0 + nt - 1] = True
            if w % BLK == 0:
                blk_first[t0] = True
            if w % BLK == BLK - 1 or w == NW - 1:
                blk_last[t0 + nt - 1] = True
    # chunk segment boundaries (in idx positions)
    chunk_off = [int(goff[k, 0]) for k in range(NCHUNK)] + [E_PAD]

    # gather-layout idx: [16, E_PAD/16] with [p, s] = stream[s*16+p]
    idx_wrapped = np.ascontiguousarray(
        idx16.reshape(C, E_PAD // 16, 16).transpose(0, 2, 1))
    # dstl layout [128, E_PAD/128] with [p, t] = stream[t*128+p]
    dstl_wrapped = np.ascontiguousarray(
        dstl.reshape(C, TILES, P).transpose(0, 2, 1))

    # per-core padded node arrays
    deg_pad = np.ones((C, S), dtype=np.float32)
    batchl = np.full((C, S), float(GMAX - 1), dtype=np.float32)
    for c in range(C):
        n0, n1 = int(node_bnds[c]), int(node_bnds[c + 1])
        deg_pad[c, : n1 - n0] = deg[n0:n1]
        batchl[c, : n1 - n0] = (batch[n0:n1] - g_bnds[c]).astype(np.float32)
    batchl_wrapped = np.ascontiguousarray(
        batchl.reshape(C, NW, P).transpose(0, 2, 1))         # [C, 128, NW]

    return dict(
        S=S, NW=NW, E_PAD=E_PAD, TILES=TILES,
        node_bnds=node_bnds, g_bnds=g_bnds, node_cnt=node_cnt, g_cnt=g_cnt,
        idx_wrapped=idx_wrapped, dstl_wrapped=dstl_wrapped,
        batchl_wrapped=batchl_wrapped, deg_pad=deg_pad,
        tile_win=tile_win, tile_first=tile_first, tile_last=tile_last,
        blk_first=blk_first, blk_last=blk_last,
        chunk_off=chunk_off,
    )


# ----------------------------------------------------------------------------
# Bass program builder
# ----------------------------------------------------------------------------

def _build_program(meta, d_in, h_dim, n_cls):
    import concourse.bacc as bacc
    import concourse.mybir as mybir
    import concourse.tile as tile
    from concourse import library_config

    S, NW, E_PAD = meta["S"], meta["NW"], meta["E_PAD"]
    tile_win = meta["tile_win"]
    tile_first = meta["tile_first"]
    tile_last = meta["tile_last"]
    blk_first = meta["blk_first"]
    blk_last = meta["blk_last"]
    chunk_off = meta["chunk_off"]
    f32 = mybir.dt.float32
    bf16 = mybir.dt.bfloat16
    AOT = mybir.ActivationFunctionType
    ALU = mybir.AluOpType

    nc = bacc.Bacc("TRN2", target_bir_lowering=False, debug=False,
                   num_devices=N_CORES, num_swdge_queues=4)

    # --- I/O ---
    x_d = nc.dram_tensor("x_loc", [S, d_in], f32, kind="ExternalInput").ap()
    deg_d = nc.dram_tensor("deg_loc", [S], f32, kind="ExternalInput").ap()
    idx_d = nc.dram_tensor("idx16", [P, E_PAD // 16], mybir.dt.int16,
                           kind="ExternalInput").ap()
    dstl_d = nc.dram_tensor("dstl", [P, E_PAD // P], bf16,
                            kind="ExternalInput").ap()
    batchl_d = nc.dram_tensor("batchl", [P, NW], f32, kind="ExternalInput").ap()
    W_d = [nc.dram_tensor("W1", [d_in, h_dim], f32, kind="ExternalInput").ap(),
           nc.dram_tensor("W2", [h_dim, h_dim], f32, kind="ExternalInput").ap(),
           nc.dram_tensor("W3", [h_dim, h_dim], f32, kind="ExternalInput").ap()]
    Wf_d = nc.dram_tensor("Wf", [h_dim, n_cls], f32, kind="ExternalInput").ap()
    b_d = [nc.dram_tensor(f"b{i+1}b", [P, h_dim], f32, kind="ExternalInput").ap()
           for i in range(3)]
    bf_d = nc.dram_tensor("bfb", [P, n_cls], f32, kind="ExternalInput").ap()
    out_d = nc.dram_tensor("out", [GMAX, n_cls], f32, kind="ExternalOutput").ap()
    ident_d = nc.dram_tensor("ident", [P, P], f32, kind="ExternalInput").ap()
    iota_w_d = nc.dram_tensor("iota_w", [P, WIN], bf16, kind="ExternalInput").ap()
    iota_g_d = nc.dram_tensor("iota_g", [P, GMAX], f32, kind="ExternalInput").ap()

    S2 = S // 2
    znA_d = nc.dram_tensor("znA_loc", [S2, h_dim], f32).ap()
    znB_d = nc.dram_tensor("znB_loc", [S2, h_dim], f32).ap()
    tableA_d = nc.dram_tensor("tableA", [N_CORES * S2, h_dim], f32,
                              addr_space="Shared").ap()
    tableB_d = nc.dram_tensor("tableB", [N_CORES * S2, h_dim], f32,
                              addr_space="Shared").ap()
    cvA = tableA_d.rearrange("(n two) d -> two n d", two=2)
    cvB = tableB_d.rearrange("(n two) d -> two n d", two=2)
    chunk_views = [cvA[0], cvA[1], cvB[0], cvB[1]]
    rg = [list(range(N_CORES))]

    with tile.TileContext(nc) as tc:
        with (
            tc.tile_pool(name="persist", bufs=1) as pp,
            tc.tile_pool(name="msg", bufs=8) as msgp,
            tc.tile_pool(name="ohp", bufs=2) as ohp,
            tc.tile_pool(name="work", bufs=4) as wp,
            tc.tile_pool(name="dense", bufs=3) as dp,
            tc.tile_pool(name="psum", bufs=2, space="PSUM") as psp,
            tc.tile_pool(name="psum1", bufs=2, space="PSUM") as ps1,
            tc.tile_pool(name="pool_ps", bufs=1, space="PSUM") as poolps,
        ):
            # --- persistent tiles ---
            nc.gpsimd.load_library(library_config.mlp)
            ident = pp.tile([P, P], f32, tag="ident")
            nc.sync.dma_start(ident[:], ident_d[:])
            iota_w = pp.tile([P, WIN], bf16, tag="iota_w")
            nc.sync.dma_start(iota_w[:], iota_w_d[:])
            iota_g = pp.tile([P, GMAX], f32, tag="iota_g")
            nc.sync.dma_start(iota_g[:], iota_g_d[:])

            W_sb = []
            for i in range(3):
                k = d_in if i == 0 else h_dim
                t = pp.tile([k, h_dim], f32, tag=f"W{i}")
                nc.sync.dma_start(t[:], W_d[i][:])
                W_sb.append(t)
            Wf_sb = pp.tile([h_dim, n_cls], f32, tag="Wf")
            nc.sync.dma_start(Wf_sb[:], Wf_d[:])
            b_sb = []
            for i in range(3):
                t = pp.tile([P, h_dim], f32, tag=f"b{i}")
                nc.sync.dma_start(t[:], b_d[i][:])
                b_sb.append(t)
            bf_sb = pp.tile([P, n_cls], f32, tag="bf")
            nc.sync.dma_start(bf_sb[:], bf_d[:])

            idx_sb = pp.tile([P, E_PAD // 16], mybir.dt.int16, tag="idx")
            nc.sync.dma_start(idx_sb[:], idx_d[:])
            dstl_sb = pp.tile([P, E_PAD // P], bf16, tag="dstl")
            nc.sync.dma_start(dstl_sb[:], dstl_d[:])
            batchl_sb = pp.tile([P, NW], f32, tag="batchl")
            nc.sync.dma_start(batchl_sb[:], batchl_d[:])

            dinv = pp.tile([P, NW], f32, tag="dinv")
            deg_col = pp.tile([P, NW], f32, tag="degc")
            nc.sync.dma_start(deg_col[:], deg_d.rearrange("(t p) -> p t", p=P))
            # dinv = 1/sqrt(deg): sqrt on ACT, then DVE reciprocal
            nc.scalar.activation(deg_col[:], deg_col[:], AOT.Sqrt)
            nc.vector.reciprocal(dinv[:], deg_col[:])

            bufA = pp.tile([P, NW * h_dim], f32, tag="bufA")

            bufA3 = bufA[:].rearrange("p (w d) -> p w d", d=h_dim)
            dinv_bc = dinv[:].unsqueeze(2).broadcast_to([P, NW, h_dim])

            # === 3 GCN layers ===
            for layer in range(3):
                # ---- dense: z = h_in @ W, 8 tiles per psum bank ----
                zbank = None
                for t in range(NW):
                    if layer == 0:
                        xt = dp.tile([P, d_in], f32, tag="xt")
                        nc.sync.dma_start(xt[:], x_d[t * P:(t + 1) * P, :])
                        tp = ps1.tile([d_in, P], f32, tag="tps")
                        nc.tensor.transpose(tp[:], xt[:], ident[:])
                        sbT = dp.tile([d_in, P], f32, tag="sbT")
                        nc.vector.tensor_copy(sbT[:], tp[:])
                    else:
                        tp = ps1.tile([h_dim, P], f32, tag="tps")
                        nc.tensor.transpose(
                            tp[:], bufA[:, t * h_dim:(t + 1) * h_dim], ident[:])
                        sbT = dp.tile([h_dim, P], f32, tag="sbT")
                        nc.vector.tensor_copy(sbT[:], tp[:])
                    if t % BLK == 0:
                        zbank = psp.tile([P, BLK * h_dim], f32, tag="zbank")
                    zsl = zbank[:, (t % BLK) * h_dim:(t % BLK + 1) * h_dim]
                    nc.tensor.matmul(zsl, lhsT=sbT[:], rhs=W_sb[layer][:],
                                     start=True, stop=True)
                    if t % BLK == BLK - 1 or t == NW - 1:
                        t0 = (t // BLK) * BLK
                        nc.vector.tensor_copy(
                            bufA[:, t0 * h_dim:(t + 1) * h_dim],
                            zbank[:, :(t - t0 + 1) * h_dim])
                # zn = z * dinv, one batched instr
                nc.vector.tensor_tensor(out=bufA3, in0=bufA3, in1=dinv_bc,
                                        op=ALU.mult)

                # ---- publish zn halves + 2 AllGathers (B overlaps the
                #      chunk-0/1 gather stretch, which reads tableA only) ----
                NW2 = NW // 2
                bufA_v = bufA[:].rearrange("p (t d) -> p t d", d=h_dim)
                nc.sync.dma_start(
                    znA_d.rearrange("(t p) d -> p t d", p=P),
                    bufA_v[:, :NW2, :])
                nc.sync.dma_start(
                    znB_d.rearrange("(t p) d -> p t d", p=P),
                    bufA_v[:, NW2:, :])
                nc.gpsimd.collective_compute(
                    "AllGather", ALU.bypass, replica_groups=rg,
                    ins=[znA_d[:]], outs=[tableA_d[:]])
                nc.gpsimd.collective_compute(
                    "AllGather", ALU.bypass, replica_groups=rg,
                    ins=[znB_d[:]], outs=[tableB_d[:]])

                # ---- sparse aggregation: 4-queue gathers + batched one-hot
                #      gen + psum-block matmul accumulation ----
                wpsum = None
                qcnt = 0
                for k in range(NCHUNK):
                    seg0, seg1 = chunk_off[k], chunk_off[k + 1]
                    for a in range(seg0, seg1, GRANULE):
                        gsz = min(GRANULE, seg1 - a)
                        gT = gsz // P
                        msg = msgp.tile([P, gT * h_dim], f32, tag="msg")
                        nc.gpsimd.dma_gather(
                            msg[:].rearrange("p (t d) -> p t d", d=h_dim),
                            chunk_views[k],
                            idx_sb[:, a // 16:(a + gsz) // 16],
                            gsz, gsz, h_dim, elem_step=2 * h_dim,
                            single_packet=False, queue_num=qcnt % 4)
                        qcnt += 1
                        t0g = a // P
                        msg16 = msgp.tile([P, gT * h_dim], bf16, tag="msg16")
                        nc.scalar.copy(msg16[:], msg[:])
                        ohblk = ohp.tile([P, gT * WIN], bf16, tag="ohblk")
                        nc.vector.tensor_tensor(
                            out=ohblk[:].rearrange("p (t w) -> p t w", w=WIN),
                            in0=iota_w[:].unsqueeze(1).broadcast_to(
                                [P, gT, WIN]),
                            in1=dstl_sb[:, t0g:t0g + gT].unsqueeze(2)
                                .broadcast_to([P, gT, WIN]),
                            op=ALU.is_equal)
                        for i in range(gT):
                            t = t0g + i
                            w = int(tile_win[t])
                            if blk_first[t]:
                                wpsum = psp.tile([P, BLK * h_dim], f32,
                                                 tag="wps")
                            nc.tensor.matmul(
                                wpsum[:, (w % BLK) * h_dim:
                                      (w % BLK + 1) * h_dim],
                                lhsT=ohblk[:, i * WIN:(i + 1) * WIN],
                                rhs=msg16[:, i * h_dim:(i + 1) * h_dim],
                                start=bool(tile_first[t]),
                                stop=bool(tile_last[t]))
                            if blk_last[t]:
                                w0 = (w // BLK) * BLK
                                dst = bufA[:, w0 * h_dim:(w + 1) * h_dim]
                                src = wpsum[:, :(w - w0 + 1) * h_dim]
                                if k == 0:
                                    nc.vector.tensor_copy(dst, src)
                                else:
                                    nc.vector.tensor_tensor(
                                        out=dst, in0=dst, in1=src, op=ALU.add)

                # ---- flush: h = tanh(dinv * acc + b), batched in place ----
                nc.vector.tensor_tensor(out=bufA3, in0=bufA3, in1=dinv_bc,
                                        op=ALU.mult)
                nc.vector.tensor_tensor(
                    out=bufA3, in0=bufA3,
                    in1=b_sb[layer][:].unsqueeze(1).broadcast_to(
                        [P, NW, h_dim]),
                    op=ALU.add)
                nc.scalar.activation(bufA[:], bufA[:], AOT.Tanh)

            # === pooling: pooledT[64, GMAX] = sum_h3 by graph ===
            poolT = poolps.tile([h_dim, GMAX], f32, tag="poolT")
            NBW = -(-NW // BLK)
            ohg_blk = pp.tile([P, BLK * GMAX], f32, tag="ohgblk")
            for bw in range(NBW):
                w0 = bw * BLK
                nw = min(BLK, NW - w0)
                nc.vector.tensor_tensor(
                    out=ohg_blk[:, :nw * GMAX].rearrange(
                        "p (t g) -> p t g", g=GMAX),
                    in0=iota_g[:].unsqueeze(1).broadcast_to([P, nw, GMAX]),
                    in1=batchl_sb[:, w0:w0 + nw].unsqueeze(2)
                        .broadcast_to([P, nw, GMAX]),
                    op=ALU.is_equal)
                for i in range(nw):
                    t = w0 + i
                    nc.tensor.matmul(
                        poolT[:],
                        lhsT=bufA[:, t * h_dim:(t + 1) * h_dim],
                        rhs=ohg_blk[:, i * GMAX:(i + 1) * GMAX],
                        start=(t == 0), stop=(t == NW - 1))
            poolS = pp.tile([h_dim, GMAX], f32, tag="poolS")
            nc.vector.tensor_copy(poolS[:], poolT[:])

            # === head: out = tanh(pooled @ Wf + bf) ===
            for gt in range(GMAX // P):
                fps = psp.tile([P, n_cls], f32, tag="wps")
                nc.tensor.matmul(fps[:], lhsT=poolS[:, gt * P:(gt + 1) * P],
                                 rhs=Wf_sb[:], start=True, stop=True)
                ot = wp.tile([P, n_cls], f32, tag="ot")
                nc.vector.tensor_tensor(out=ot[:], in0=fps[:], in1=bf_sb[:],
                                        op=ALU.add)
                nc.scalar.activation(ot[:], ot[:], AOT.Tanh)
                nc.sync.dma_start(out_d[gt * P:(gt + 1) * P, :], ot[:])

    nc.compile()
    return nc


# ----------------------------------------------------------------------------
# Runner (persistent compiled program + per-core inputs)
# ----------------------------------------------------------------------------

class Runner:
    def __init__(self, meta, nc, d_in, h_dim, n_cls):
        self.meta = meta
        self.nc = nc
        self.d_in, self.h_dim, self.n_cls = d_in, h_dim, n_cls

    def in_maps(self, x, W1, b1, W2, b2, W3, b3, Wf, bf):
        _bf = _bfnp()
        m = self.meta
        S = m["S"]
        C = N_CORES
        x = np.asarray(x, np.float32)
        maps = []
        reps = dict(
            W1=np.asarray(W1, np.float32), W2=np.asarray(W2, np.float32),
            W3=np.asarray(W3, np.float32), Wf=np.asarray(Wf, np.float32),
            b1b=np.broadcast_to(np.asarray(b1, np.float32), (P, self.h_dim)).copy(),
            b2b=np.broadcast_to(np.asarray(b2, np.float32), (P, self.h_dim)).copy(),
            b3b=np.broadcast_to(np.asarray(b3, np.float32), (P, self.h_dim)).copy(),
            bfb=np.broadcast_to(np.asarray(bf, np.float32), (P, self.n_cls)).copy(),
            ident=np.eye(P, dtype=np.float32),
            iota_w=np.broadcast_to(np.arange(WIN, dtype=np.float32), (P, WIN)).astype(_bf), 
            iota_g=np.broadcast_to(np.arange(GMAX, dtype=np.float32), (P, GMAX)).copy(),
        )
        for c in range(C):
            n0, n1 = int(m["node_bnds"][c]), int(m["node_bnds"][c + 1])
            xl = np.zeros((S, self.d_in), np.float32)
            xl[: n1 - n0] = x[n0:n1]
            maps.append(dict(
                x_loc=xl,
                deg_loc=m["deg_pad"][c],
                idx16=np.tile(m["idx_wrapped"][c], (8, 1)),
                dstl=m["dstl_wrapped"][c].astype(_bf),
                batchl=m["batchl_wrapped"][c],
                **reps,
            ))
        return maps

    def run(self, maps):
        from concourse.bass_utils import run_bass_kernel_spmd
        res = run_bass_kernel_spmd(self.nc, maps, list(range(N_CORES)))
        return self.assemble(res.results)

    def make_timed(self, maps):
        """Build a callable with inputs resident on device; each call runs the
        NEFF once and returns per-core outputs. For timing (transfer excluded)."""
        import jax
        import concourse.mybir as mybir
        from concourse import bass2jax
        from jax.experimental.shard_map import shard_map
        from jax.sharding import Mesh, NamedSharding, PartitionSpec

        nc = self.nc
        bass2jax.install_neuronx_cc_hook()
        partition_name = (nc.partition_id_tensor.name
                          if nc.partition_id_tensor else None)
        in_names, out_names, out_avals, zero_outs = [], [], [], []
        for alloc in nc.m.functions[0].allocations:
            if not isinstance(alloc, mybir.MemoryLocationSet):
                continue
            name = alloc.memorylocations[0].name
            if alloc.kind == "ExternalInput":
                if name != partition_name:
                    in_names.append(name)
            elif alloc.kind == "ExternalOutput":
                shape = tuple(alloc.tensor_shape)
                dtype = mybir.dt.np(alloc.dtype)
                out_names.append(name)
                out_avals.append(jax.core.ShapedArray(shape, dtype))
                zero_outs.append(np.zeros(shape, dtype))
        n_params = len(in_names)
        all_in = list(in_names) + list(out_names)
        if partition_name is not None:
            all_in.append(partition_name)
        donate = tuple(range(n_params, n_params + len(out_names)))

        def _body(*args):
            operands = list(args)
            if partition_name is not None:
                operands.append(bass2jax.partition_id_tensor())
            return tuple(bass2jax._bass_exec_p.bind(
                *operands, out_avals=tuple(out_avals), in_names=tuple(all_in),
                out_names=tuple(out_names), lowering_input_output_aliases=(),
                sim_require_finite=True, sim_require_nnan=True, nc=nc))

        devices = jax.devices()[:N_CORES]
        mesh = Mesh(np.asarray(devices), ("core",))
        spec = NamedSharding(mesh, PartitionSpec("core"))
        fn = jax.jit(shard_map(_body, mesh=mesh,
                               in_specs=(PartitionSpec("core"),) * (n_params + len(out_names)),
                               out_specs=(PartitionSpec("core"),) * len(out_names)),
                     donate_argnums=donate, keep_unused=True)
        dev_in = [jax.device_put(
            np.concatenate([np.asarray(maps[c][nm]) for c in range(N_CORES)], axis=0),
            spec) for nm in in_names]
        zshapes = [(N_CORES * z.shape[0], *z.shape[1:]) for z in zero_outs]
        zdtypes = [z.dtype for z in zero_outs]

        def call():
            zs = [jax.device_put(np.zeros(s, d), spec)
                  for s, d in zip(zshapes, zdtypes)]
            outs = fn(*dev_in, *zs)
            return [o.block_until_ready() for o in outs]

        return call, out_names, out_avals

    def assemble(self, results):
        m = self.meta
        outs = []
        for c in range(N_CORES):
            outs.append(results[c]["out"][: int(m["g_cnt"][c])])
        return np.concatenate(outs, axis=0)


_CACHE = {}


def _get_runner(edge_index, batch, n_nodes, n_graphs, d_in, h_dim, n_cls):
    key = (hashlib.sha1(np.ascontiguousarray(edge_index).tobytes()).hexdigest(),
           hashlib.sha1(np.ascontiguousarray(batch).tobytes()).hexdigest(),
           n_nodes, n_graphs, d_in, h_dim, n_cls)
    r = _CACHE.get(key)
    if r is None:
        meta = _preprocess(edge_index, batch, n_nodes, n_graphs)
        nc = _build_program(meta, d_in, h_dim, n_cls)
        r = Runner(meta, nc, d_in, h_dim, n_cls)
        _CACHE[key] = r
    return r


def kernel(x, edge_index, batch, W1, b1, W2, b2, W3, b3, Wf, bf):
    x = np.asarray(x)
    r = _get_runner(np.asarray(edge_index), np.asarray(batch), x.shape[0],
                    N_GRAPHS, x.shape[1], np.asarray(W1).shape[1],
                    np.asarray(Wf).shape[1])
    maps = r.in_maps(x, W1, b1, W2, b2, W3, b3, Wf, bf)
    return r.run(maps)



# revision 24
# speedup vs baseline: 870.3326x; 1.0375x over previous
"""GCN (3-layer GCNConv + global_add_pool + linear head) on 8 Trainium2 cores.

Strategy:
 - Nodes sharded across 8 cores on graph-id-aligned boundaries (pooling local).
 - Edges partitioned by dst owner. Per core, edges ordered chunk-major
   (src_row % 4 -> int16 gather index fits), then by 128-node dst window,
   padded per (chunk, window) group to multiples of 128 and uniform tile
   counts across cores (single SPMD program).
 - Per layer: dense z = h @ W on PE (transpose-on-the-fly, 8 z-tiles batched
   per PSUM bank), one batched zn = z * dinv, two half-table AllGathers
   (chunks 0/1 read half A so their gathers overlap AllGather-B),
   dma_gather 256B rows per edge round-robined over the 4 SWDGE queues
   (concurrent Q7 descriptor-gen + deeper DMA pipelining), per-granule
   batched one-hot generation (single broadcast-AP is_equal -> bf16),
   ACT-engine msg cast f32->bf16, segment-sum via bf16 one-hot matmuls
   accumulated into 8-window PSUM banks, per-bank flush into SBUF across the
   4 chunk passes, then one batched tanh(dinv*acc + b) per layer.
 - Pooling: batched one-hot(graph id) generation + matmul into a [64, 512]
   PSUM tile; final linear head + tanh on device.
 - Instruction-count economics are the main lever: one-hot gen, scaling,
   bias, tanh and psum flushes are all batched (engine sequencers otherwise
   serialize at ~0.3-0.9us per instruction).
"""

import hashlib
import sys

for _p in ("/opt/trn_rl_repo",):
    if _p not in sys.path:
        sys.path.insert(0, _p)

import numpy as np

def _bfnp():
    import concourse.mybir as mybir
    return mybir.dt.np(mybir.dt.bfloat16)

P = 128
WIN = 128          # dst-window width (nodes)
NCHUNK = 4         # src chunks (int16 index limit: 8S/4 <= 32767)
GRANULE = 4096     # idxs per dma_gather instruction
N_CORES = 8
N_GRAPHS = 2048    # problem constant
GMAX = 512         # per-core graph-count upper bound (psum free dim)
BLK = 8            # windows per psum accumulation bank (512 f32)


# ----------------------------------------------------------------------------
# Host-side sharding / edge bucketing (index manipulation only, no float math)
# ----------------------------------------------------------------------------

def _preprocess(edge_index, batch, n_nodes, n_graphs):
    C = N_CORES
    src = np.asarray(edge_index[0], dtype=np.int64)
    dst = np.asarray(edge_index[1], dtype=np.int64)
    batch = np.asarray(batch, dtype=np.int64)
    N = n_nodes

    # graph-aligned node shard boundaries
    gstart = np.searchsorted(batch, np.arange(n_graphs + 1))  # [G+1], gstart[G] = N
    node_bnds = [0]
    g_bnds = [0]
    for c in range(1, C):
        tgt = (c * N) // C
        g = int(np.searchsorted(gstart, tgt))
        # candidates g-1, g: pick nearest boundary node
        if g > 0 and abs(int(gstart[g - 1]) - tgt) <= abs(int(gstart[min(g, n_graphs)]) - tgt):
            g = g - 1
        g = min(max(g, g_bnds[-1]), n_graphs)
        g_bnds.append(g)
        node_bnds.append(int(gstart[g]))
    node_bnds.append(N)
    g_bnds.append(n_graphs)
    node_bnds = np.array(node_bnds, dtype=np.int64)          # [C+1]
    g_bnds = np.array(g_bnds, dtype=np.int64)                # [C+1]
    node_cnt = node_bnds[1:] - node_bnds[:-1]
    g_cnt = g_bnds[1:] - g_bnds[:-1]
    assert g_cnt.max() < GMAX - 1, g_cnt

    S = int(-(-node_cnt.max() // P) * P)                     # padded shard size
    NW = S // WIN                                            # windows per core
    assert 2 * S <= 32767, S                                 # int16 gather idx bound

    owner = np.searchsorted(node_bnds[1:], np.arange(N), side="right")
    local = np.arange(N) - node_bnds[owner]
    row = owner * S + local                                  # table row per node

    deg = np.bincount(dst, minlength=N).astype(np.float32) + 1.0

    # edge stream (+ self loops)
    e_src = np.concatenate([src, np.arange(N)])
    e_dst = np.concatenate([dst, np.arange(N)])
    e_owner = owner[e_dst]
    e_dl = local[e_dst]
    S2 = S // 2
    e_loc = local[e_src]
    e_osrc = owner[e_src]
    e_half = (e_loc >= S2).astype(np.int64)
    rh = e_osrc * S2 + (e_loc - e_half * S2)     # row within half-table
    e_chunk = (2 * e_half + (rh & 1)).astype(np.int64)
    e_idx = (rh >> 1).astype(np.int16)           # < 4*S2 = 25088
    e_win = e_dl >> 7

    key = (e_chunk * NW + e_win) * C + e_owner               # chunk-major, then window
    order = np.argsort(key, kind="stable")
    cnt = np.bincount(key, minlength=NCHUNK * NW * C).reshape(NCHUNK, NW, C)

    tiles_kw = -(-cnt.max(axis=2) // P)                      # [NCHUNK, NW] uniform tiles
    # every (chunk, window) needs >=1 tile so the 8-window psum blocks are
    # fully written before each block flush
    tiles_kw = np.maximum(tiles_kw, 1)
    pad_kw = tiles_kw * P                                    # padded group sizes
    E_PAD = int(pad_kw.sum())
    # group start offsets in the uniform stream (same for all cores)
    goff = np.zeros((NCHUNK, NW), dtype=np.int64)
    goff.flat[1:] = np.cumsum(pad_kw.flat)[:-1]

    idx16 = np.zeros((C, E_PAD), dtype=np.int16)             # pad -> idx 0 (valid row)
    dstl = np.full((C, E_PAD), -1.0, dtype=np.float32)       # pad -> -1 (one-hot miss)

    # place real edges
    so = order
    r_owner = e_owner[so]
    r_chunk = e_chunk[so]
    r_win = e_win[so]
    # position within (chunk, win, owner) group = running index
    rkey = (r_chunk * NW + r_win) * C + r_owner
    # stable sort => positions are 0..cnt-1 in order of appearance
    pos = np.zeros(len(so), dtype=np.int64)
    _, first_idx, inv = np.unique(rkey, return_index=True, return_inverse=True)
    pos = np.arange(len(so)) - first_idx[inv]
    slot = goff[r_chunk, r_win] + pos
    idx16[r_owner, slot] = e_idx[so]
    dstl[r_owner, slot] = (e_dl[so] - r_win * WIN).astype(np.float32)

    # per-tile metadata (uniform): window id, group-first, group-last,
    # 8-window psum-block first/last
    TILES = E_PAD // P
    tile_win = np.zeros(TILES, dtype=np.int64)
    tile_first = np.zeros(TILES, dtype=bool)
    tile_last = np.zeros(TILES, dtype=bool)
    blk_first = np.zeros(TILES, dtype=bool)
    blk_last = np.zeros(TILES, dtype=bool)
    for k in range(NCHUNK):
        for w in range(NW):
            t0 = goff[k, w] // P
            nt = int(tiles_kw[k, w])
            assert nt > 0
            tile_win[t0:t0 + nt] = w
            tile_first[t0] = True
            tile_last[t0 + nt - 1] = True
            if w % BLK == 0:
                blk_first[t0] = True
            if w % BLK == BLK - 1 or w == NW - 1:
                blk_last[t0 + nt - 1] = True
    # chunk segment boundaries (in idx positions)
    chunk_off = [int(goff[k, 0]) for k in range(NCHUNK)] + [E_PAD]

    # gather-layout idx: [16, E_PAD/16] with [p, s] = stream[s*16+p]
    idx_wrapped = np.ascontiguousarray(
        idx16.reshape(C, E_PAD // 16, 16).transpose(0, 2, 1))
    # dstl layout [128, E_PAD/128] with [p, t] = stream[t*128+p]
    dstl_wrapped = np.ascontiguousarray(
        dstl.reshape(C, TILES, P).transpose(0, 2, 1))

    # per-core padded node arrays
    deg_pad = np.ones((C, S), dtype=np.float32)
    batchl = np.full((C, S), float(GMAX - 1), dtype=np.float32)
    for c in range(C):
        n0, n1 = int(node_bnds[c]), int(node_bnds[c + 1])
        deg_pad[c, : n1 - n0] = deg[n0:n1]
        batchl[c, : n1 - n0] = (batch[n0:n1] - g_bnds[c]).astype(np.float32)
    batchl_wrapped = np.ascontiguousarray(
        batchl.reshape(C, NW, P).transpose(0, 2, 1))         # [C, 128, NW]

    return dict(
        S=S, NW=NW, E_PAD=E_PAD, TILES=TILES,
        node_bnds=node_bnds, g_bnds=g_bnds, node_cnt=node_cnt, g_cnt=g_cnt,
        idx_wrapped=idx_wrapped, dstl_wrapped=dstl_wrapped,
        batchl_wrapped=batchl_wrapped, deg_pad=deg_pad,
        tile_win=tile_win, tile_first=tile_first, tile_last=tile_last,
        blk_first=blk_first, blk_last=blk_last,
        chunk_off=chunk_off,
    )


# ----------------------------------------------------------------------------
# Bass program builder
# ----------------------------------------------------------------------------

def _build_program(meta, d_in, h_dim, n_cls):
    import concourse.bacc as bacc
    import concourse.mybir as mybir
    import concourse.tile as tile
    from concourse import library_config

    S, NW, E_PAD = meta["S"], meta["NW"], meta["E_PAD"]
    tile_win = meta["tile_win"]
    tile_first = meta["tile_first"]
    tile_last = meta["tile_last"]
    blk_first = meta["blk_first"]
    blk_last = meta["blk_last"]
    chunk_off = meta["chunk_off"]
    f32 = mybir.dt.float32
    bf16 = mybir.dt.bfloat16
    AOT = mybir.ActivationFunctionType
    ALU = mybir.AluOpType

    nc = bacc.Bacc("TRN2", target_bir_lowering=False, debug=False,
                   num_devices=N_CORES, num_swdge_queues=4)

    # --- I/O ---
    x_d = nc.dram_tensor("x_loc", [S, d_in], f32, kind="ExternalInput").ap()
    deg_d = nc.dram_tensor("deg_loc", [S], f32, kind="ExternalInput").ap()
    idx_d = nc.dram_tensor("idx16", [P, E_PAD // 16], mybir.dt.int16,
                           kind="ExternalInput").ap()
    dstl_d = nc.dram_tensor("dstl", [P, E_PAD // P], bf16,
                            kind="ExternalInput").ap()
    batchl_d = nc.dram_tensor("batchl", [P, NW], f32, kind="ExternalInput").ap()
    W_d = [nc.dram_tensor("W1", [d_in, h_dim], f32, kind="ExternalInput").ap(),
           nc.dram_tensor("W2", [h_dim, h_dim], f32, kind="ExternalInput").ap(),
           nc.dram_tensor("W3", [h_dim, h_dim], f32, kind="ExternalInput").ap()]
    Wf_d = nc.dram_tensor("Wf", [h_dim, n_cls], f32, kind="ExternalInput").ap()
    b_d = [nc.dram_tensor(f"b{i+1}b", [P, h_dim], f32, kind="ExternalInput").ap()
           for i in range(3)]
    bf_d = nc.dram_tensor("bfb", [P, n_cls], f32, kind="ExternalInput").ap()
    out_d = nc.dram_tensor("out", [GMAX, n_cls], f32, kind="ExternalOutput").ap()
    ident_d = nc.dram_tensor("ident", [P, P], f32, kind="ExternalInput").ap()
    iota_w_d = nc.dram_tensor("iota_w", [P, WIN], bf16, kind="ExternalInput").ap()
    iota_g_d = nc.dram_tensor("iota_g", [P, GMAX], f32, kind="ExternalInput").ap()

    S2 = S // 2
    znA_d = nc.dram_tensor("znA_loc", [S2, h_dim], f32).ap()
    znB_d = nc.dram_tensor("znB_loc", [S2, h_dim], f32).ap()
    tableA_d = nc.dram_tensor("tableA", [N_CORES * S2, h_dim], f32,
                              addr_space="Shared").ap()
    tableB_d = nc.dram_tensor("tableB", [N_CORES * S2, h_dim], f32,
                              addr_space="Shared").ap()
    cvA = tableA_d.rearrange("(n two) d -> two n d", two=2)
    cvB = tableB_d.rearrange("(n two) d -> two n d", two=2)
    chunk_views = [cvA[0], cvA[1], cvB[0], cvB[1]]
    rg = [list(range(N_CORES))]

    with tile.TileContext(nc) as tc:
        with (
            tc.tile_pool(name="persist", bufs=1) as pp,
            tc.tile_pool(name="msg", bufs=8) as msgp,
            tc.tile_pool(name="ohp", bufs=2) as ohp,
            tc.tile_pool(name="work", bufs=4) as wp,
            tc.tile_pool(name="dense", bufs=3) as dp,
            tc.tile_pool(name="psum", bufs=2, space="PSUM") as psp,
            tc.tile_pool(name="psum1", bufs=2, space="PSUM") as ps1,
            tc.tile_pool(name="pool_ps", bufs=1, space="PSUM") as poolps,
        ):
            # --- persistent tiles ---
            nc.gpsimd.load_library(library_config.mlp)
            ident = pp.tile([P, P], f32, tag="ident")
            nc.sync.dma_start(ident[:], ident_d[:])
            iota_w = pp.tile([P, WIN], bf16, tag="iota_w")
            nc.sync.dma_start(iota_w[:], iota_w_d[:])
            iota_g = pp.tile([P, GMAX], f32, tag="iota_g")
            nc.sync.dma_start(iota_g[:], iota_g_d[:])

            W_sb = []
            for i in range(3):
                k = d_in if i == 0 else h_dim
                t = pp.tile([k, h_dim], f32, tag=f"W{i}")
                nc.sync.dma_start(t[:], W_d[i][:])
                W_sb.append(t)
            Wf_sb = pp.tile([h_dim, n_cls], f32, tag="Wf")
            nc.sync.dma_start(Wf_sb[:], Wf_d[:])
            b_sb = []
            for i in range(3):
                t = pp.tile([P, h_dim], f32, tag=f"b{i}")
                nc.sync.dma_start(t[:], b_d[i][:])
                b_sb.append(t)
            bf_sb = pp.tile([P, n_cls], f32, tag="bf")
            nc.sync.dma_start(bf_sb[:], bf_d[:])

            idx_sb = pp.tile([P, E_PAD // 16], mybir.dt.int16, tag="idx")
            nc.sync.dma_start(idx_sb[:], idx_d[:])
            dstl_sb = pp.tile([P, E_PAD // P], bf16, tag="dstl")
            nc.sync.dma_start(dstl_sb[:], dstl_d[:])
            batchl_sb = pp.tile([P, NW], f32, tag="batchl")
            nc.sync.dma_start(batchl_sb[:], batchl_d[:])

            dinv = pp.tile([P, NW], f32, tag="dinv")
            deg_col = pp.tile([P, NW], f32, tag="degc")
            nc.sync.dma_start(deg_col[:], deg_d.rearrange("(t p) -> p t", p=P))
            # dinv = 1/sqrt(deg): sqrt on ACT, then DVE reciprocal
            nc.scalar.activation(deg_col[:], deg_col[:], AOT.Sqrt)
            nc.vector.reciprocal(dinv[:], deg_col[:])

            bufA = pp.tile([P, NW * h_dim], f32, tag="bufA")

            bufA3 = bufA[:].rearrange("p (w d) -> p w d", d=h_dim)
            dinv_bc = dinv[:].unsqueeze(2).broadcast_to([P, NW, h_dim])

            # === 3 GCN layers ===
            for layer in range(3):
                # ---- dense: z = h_in @ W, 8 tiles per psum bank ----
                zbank = None
                TB = 4
                kdim = d_in if layer == 0 else h_dim
                tbank = None
                for t in range(NW):
                    if t % TB == 0:
                        tbank = ps1.tile([kdim, TB * P], f32, tag="tbank")
                    tsl = tbank[:, (t % TB) * P:(t % TB + 1) * P]
                    if layer == 0:
                        xt = dp.tile([P, d_in], f32, tag="xt")
                        nc.sync.dma_start(xt[:], x_d[t * P:(t + 1) * P, :])
                        nc.tensor.transpose(tsl, xt[:], ident[:])
                    else:
                        nc.tensor.transpose(
                            tsl, bufA[:, t * h_dim:(t + 1) * h_dim], ident[:])
                    if t % TB == TB - 1 or t == NW - 1:
                        t0b = (t // TB) * TB
                        n = t - t0b + 1
                        sbT = dp.tile([kdim, TB * P], f32, tag="sbT")
                        nc.vector.tensor_copy(sbT[:, :n * P], tbank[:, :n * P])
                        for i in range(n):
                            tt = t0b + i
                            if tt % BLK == 0:
                                zbank = psp.tile([P, BLK * h_dim], f32,
                                                 tag="zbank")
                            zsl = zbank[:, (tt % BLK) * h_dim:
                                        (tt % BLK + 1) * h_dim]
                            nc.tensor.matmul(zsl,
                                             lhsT=sbT[:, i * P:(i + 1) * P],
                                             rhs=W_sb[layer][:],
                                             start=True, stop=True)
                            if tt % BLK == BLK - 1 or tt == NW - 1:
                                tz = (tt // BLK) * BLK
                                nc.vector.tensor_copy(
                                    bufA[:, tz * h_dim:(tt + 1) * h_dim],
                                    zbank[:, :(tt - tz + 1) * h_dim])
                # zn = z * dinv, one batched instr
                nc.vector.tensor_tensor(out=bufA3, in0=bufA3, in1=dinv_bc,
                                        op=ALU.mult)

                # ---- publish zn halves + 2 AllGathers (B overlaps the
                #      chunk-0/1 gather stretch, which reads tableA only) ----
                NW2 = NW // 2
                bufA_v = bufA[:].rearrange("p (t d) -> p t d", d=h_dim)
                nc.sync.dma_start(
                    znA_d.rearrange("(t p) d -> p t d", p=P),
                    bufA_v[:, :NW2, :])
                nc.sync.dma_start(
                    znB_d.rearrange("(t p) d -> p t d", p=P),
                    bufA_v[:, NW2:, :])
                nc.gpsimd.collective_compute(
                    "AllGather", ALU.bypass, replica_groups=rg,
                    ins=[znA_d[:]], outs=[tableA_d[:]])
                nc.gpsimd.collective_compute(
                    "AllGather", ALU.bypass, replica_groups=rg,
                    ins=[znB_d[:]], outs=[tableB_d[:]])

                # ---- sparse aggregation: 4-queue gathers + batched one-hot
                #      gen + psum-block matmul accumulation ----
                wpsum = None
                qcnt = 0
                for k in range(NCHUNK):
                    seg0, seg1 = chunk_off[k], chunk_off[k + 1]
                    for a in range(seg0, seg1, GRANULE):
                        gsz = min(GRANULE, seg1 - a)
                        gT = gsz // P
                        msg = msgp.tile([P, gT * h_dim], f32, tag="msg")
                        nc.gpsimd.dma_gather(
                            msg[:].rearrange("p (t d) -> p t d", d=h_dim),
                            chunk_views[k],
                            idx_sb[:, a // 16:(a + gsz) // 16],
                            gsz, gsz, h_dim, elem_step=2 * h_dim,
                            single_packet=False, queue_num=qcnt % 4)
                        qcnt += 1
                        t0g = a // P
                        msg16 = msgp.tile([P, gT * h_dim], bf16, tag="msg16")
                        nc.scalar.copy(msg16[:], msg[:])
                        ohblk = ohp.tile([P, gT * WIN], bf16, tag="ohblk")
                        nc.vector.tensor_tensor(
                            out=ohblk[:].rearrange("p (t w) -> p t w", w=WIN),
                            in0=iota_w[:].unsqueeze(1).broadcast_to(
                                [P, gT, WIN]),
                            in1=dstl_sb[:, t0g:t0g + gT].unsqueeze(2)
                                .broadcast_to([P, gT, WIN]),
                            op=ALU.is_equal)
                        for i in range(gT):
                            t = t0g + i
                            w = int(tile_win[t])
                            if blk_first[t]:
                                wpsum = psp.tile([P, BLK * h_dim], f32,
                                                 tag="wps")
                            nc.tensor.matmul(
                                wpsum[:, (w % BLK) * h_dim:
                                      (w % BLK + 1) * h_dim],
                                lhsT=ohblk[:, i * WIN:(i + 1) * WIN],
                                rhs=msg16[:, i * h_dim:(i + 1) * h_dim],
                                start=bool(tile_first[t]),
                                stop=bool(tile_last[t]))
                            if blk_last[t]:
                                w0 = (w // BLK) * BLK
                                dst = bufA[:, w0 * h_dim:(w + 1) * h_dim]
                                src = wpsum[:, :(w - w0 + 1) * h_dim]
                                if k == 0:
                                    nc.vector.tensor_copy(dst, src)
                                else:
                                    nc.vector.tensor_tensor(
                                        out=dst, in0=dst, in1=src, op=ALU.add)

                # ---- flush: h = tanh(dinv * acc + b), batched in place ----
                nc.vector.tensor_tensor(out=bufA3, in0=bufA3, in1=dinv_bc,
                                        op=ALU.mult)
                nc.vector.tensor_tensor(
                    out=bufA3, in0=bufA3,
                    in1=b_sb[layer][:].unsqueeze(1).broadcast_to(
                        [P, NW, h_dim]),
                    op=ALU.add)
                nc.scalar.activation(bufA[:], bufA[:], AOT.Tanh)

            # === pooling: pooledT[64, GMAX] = sum_h3 by graph ===
            poolT = poolps.tile([h_dim, GMAX], f32, tag="poolT")
            NBW = -(-NW // BLK)
            ohg_blk = pp.tile([P, BLK * GMAX], f32, tag="ohgblk")
            for bw in range(NBW):
                w0 = bw * BLK
                nw = min(BLK, NW - w0)
                nc.vector.tensor_tensor(
                    out=ohg_blk[:, :nw * GMAX].rearrange(
                        "p (t g) -> p t g", g=GMAX),
                    in0=iota_g[:].unsqueeze(1).broadcast_to([P, nw, GMAX]),
                    in1=batchl_sb[:, w0:w0 + nw].unsqueeze(2)
                        .broadcast_to([P, nw, GMAX]),
                    op=ALU.is_equal)
                for i in range(nw):
                    t = w0 + i
                    nc.tensor.matmul(
                        poolT[:],
                        lhsT=bufA[:, t * h_dim:(t + 1) * h_dim],
                        rhs=ohg_blk[:, i * GMAX:(i + 1) * GMAX],
                        start=(t == 0), stop=(t == NW - 1))
            poolS = pp.tile([h_dim, GMAX], f32, tag="poolS")
            nc.vector.tensor_copy(poolS[:], poolT[:])

            # === head: out = tanh(pooled @ Wf + bf) ===
            for gt in range(GMAX // P):
                fps = psp.tile([P, n_cls], f32, tag="wps")
                nc.tensor.matmul(fps[:], lhsT=poolS[:, gt * P:(gt + 1) * P],
                                 rhs=Wf_sb[:], start=True, stop=True)
                ot = wp.tile([P, n_cls], f32, tag="ot")
                nc.vector.tensor_tensor(out=ot[:], in0=fps[:], in1=bf_sb[:],
                                        op=ALU.add)
                nc.scalar.activation(ot[:], ot[:], AOT.Tanh)
                nc.sync.dma_start(out_d[gt * P:(gt + 1) * P, :], ot[:])

    nc.compile()
    return nc


# ----------------------------------------------------------------------------
# Runner (persistent compiled program + per-core inputs)
# ----------------------------------------------------------------------------

class Runner:
    def __init__(self, meta, nc, d_in, h_dim, n_cls):
        self.meta = meta
        self.nc = nc
        self.d_in, self.h_dim, self.n_cls = d_in, h_dim, n_cls

    def in_maps(self, x, W1, b1, W2, b2, W3, b3, Wf, bf):
        _bf = _bfnp()
        m = self.meta
        S = m["S"]
        C = N_CORES
        x = np.asarray(x, np.float32)
        maps = []
        reps = dict(
            W1=np.asarray(W1, np.float32), W2=np.asarray(W2, np.float32),
            W3=np.asarray(W3, np.float32), Wf=np.asarray(Wf, np.float32),
            b1b=np.broadcast_to(np.asarray(b1, np.float32), (P, self.h_dim)).copy(),
            b2b=np.broadcast_to(np.asarray(b2, np.float32), (P, self.h_dim)).copy(),
            b3b=np.broadcast_to(np.asarray(b3, np.float32), (P, self.h_dim)).copy(),
            bfb=np.broadcast_to(np.asarray(bf, np.float32), (P, self.n_cls)).copy(),
            ident=np.eye(P, dtype=np.float32),
            iota_w=np.broadcast_to(np.arange(WIN, dtype=np.float32), (P, WIN)).astype(_bf), 
            iota_g=np.broadcast_to(np.arange(GMAX, dtype=np.float32), (P, GMAX)).copy(),
        )
        for c in range(C):
            n0, n1 = int(m["node_bnds"][c]), int(m["node_bnds"][c + 1])
            xl = np.zeros((S, self.d_in), np.float32)
            xl[: n1 - n0] = x[n0:n1]
            maps.append(dict(
                x_loc=xl,
                deg_loc=m["deg_pad"][c],
                idx16=np.tile(m["idx_wrapped"][c], (8, 1)),
                dstl=m["dstl_wrapped"][c].astype(_bf),
                batchl=m["batchl_wrapped"][c],
                **reps,
            ))
        return maps

    def run(self, maps):
        from concourse.bass_utils import run_bass_kernel_spmd
        res = run_bass_kernel_spmd(self.nc, maps, list(range(N_CORES)))
        return self.assemble(res.results)

    def make_timed(self, maps):
        """Build a callable with inputs resident on device; each call runs the
        NEFF once and returns per-core outputs. For timing (transfer excluded)."""
        import jax
        import concourse.mybir as mybir
        from concourse import bass2jax
        from jax.experimental.shard_map import shard_map
        from jax.sharding import Mesh, NamedSharding, PartitionSpec

        nc = self.nc
        bass2jax.install_neuronx_cc_hook()
        partition_name = (nc.partition_id_tensor.name
                          if nc.partition_id_tensor else None)
        in_names, out_names, out_avals, zero_outs = [], [], [], []
        for alloc in nc.m.functions[0].allocations:
            if not isinstance(alloc, mybir.MemoryLocationSet):
                continue
            name = alloc.memorylocations[0].name
            if alloc.kind == "ExternalInput":
                if name != partition_name:
                    in_names.append(name)
            elif alloc.kind == "ExternalOutput":
                shape = tuple(alloc.tensor_shape)
                dtype = mybir.dt.np(alloc.dtype)
                out_names.append(name)
                out_avals.append(jax.core.ShapedArray(shape, dtype))
                zero_outs.append(np.zeros(shape, dtype))
        n_params = len(in_names)
        all_in = list(in_names) + list(out_names)
        if partition_name is not None:
            all_in.append(partition_name)
        donate = tuple(range(n_params, n_params + len(out_names)))

        def _body(*args):
            operands = list(args)
            if partition_name is not None:
                operands.append(bass2jax.partition_id_tensor())
            return tuple(bass2jax._bass_exec_p.bind(
                *operands, out_avals=tuple(out_avals), in_names=tuple(all_in),
                out_names=tuple(out_names), lowering_input_output_aliases=(),
                sim_require_finite=True, sim_require_nnan=True, nc=nc))

        devices = jax.devices()[:N_CORES]
        mesh = Mesh(np.asarray(devices), ("core",))
        spec = NamedSharding(mesh, PartitionSpec("core"))
        fn = jax.jit(shard_map(_body, mesh=mesh,
                               in_specs=(PartitionSpec("core"),) * (n_params + len(out_names)),
                               out_specs=(PartitionSpec("core"),) * len(out_names)),
                     donate_argnums=donate, keep_unused=True)
        dev_in = [jax.device_put(
            np.concatenate([np.asarray(maps[c][nm]) for c in range(N_CORES)], axis=0),
            spec) for nm in in_names]
        zshapes = [(N_CORES * z.shape[0], *z.shape[1:]) for z in zero_outs]
        zdtypes = [z.dtype for z in zero_outs]

        def call():
            zs = [jax.device_put(np.zeros(s, d), spec)
                  for s, d in zip(zshapes, zdtypes)]
            outs = fn(*dev_in, *zs)
            return [o.block_until_ready() for o in outs]

        return call, out_names, out_avals

    def assemble(self, results):
        m = self.meta
        outs = []
        for c in range(N_CORES):
            outs.append(results[c]["out"][: int(m["g_cnt"][c])])
        return np.concatenate(outs, axis=0)


_CACHE = {}


def _get_runner(edge_index, batch, n_nodes, n_graphs, d_in, h_dim, n_cls):
    key = (hashlib.sha1(np.ascontiguousarray(edge_index).tobytes()).hexdigest(),
           hashlib.sha1(np.ascontiguousarray(batch).tobytes()).hexdigest(),
           n_nodes, n_graphs, d_in, h_dim, n_cls)
    r = _CACHE.get(key)
    if r is None:
        meta = _preprocess(edge_index, batch, n_nodes, n_graphs)
        nc = _build_program(meta, d_in, h_dim, n_cls)
        r = Runner(meta, nc, d_in, h_dim, n_cls)
        _CACHE[key] = r
    return r


def kernel(x, edge_index, batch, W1, b1, W2, b2, W3, b3, Wf, bf):
    x = np.asarray(x)
    r = _get_runner(np.asarray(edge_index), np.asarray(batch), x.shape[0],
                    N_GRAPHS, x.shape[1], np.asarray(W1).shape[1],
                    np.asarray(Wf).shape[1])
    maps = r.in_maps(x, W1, b1, W2, b2, W3, b3, Wf, bf)
    return r.run(maps)

